# revision 1
# baseline (speedup 1.0000x reference)
"""Multi-head causal attention (B=2, S=2048, D=2048, H=16) on 8 TRN2 NeuronCores.

Sharding (host-side): core c in 0..7 handles batch b=c//4 and heads
4*(c%4)..4*(c%4)+4 (a 512-wide column slice of wq/wk/wv, row slice of wp).
Each core computes its 4 heads' attention and a partial output projection
[S, D]; the host sums the 4 partials per batch and adds bp.

Per-core kernel (all matmuls in float32r -> full PE speed, ~2e-4 rel err):
  A) QKV projections from host-pre-transposed xT (d-major):
       qT/kT per head in [hd=128, S] layout, v per head in [S, hd] natural
       layout, written to DRAM scratch.
  B) Per head, per 512-wide q chunk j: scoresT = K_tile @ Q_chunk in
     [keys, q] layout (causal: only key tiles <= diagonal).  The causal mask
     for diagonal blocks is ADDED IN PSUM by a second matmul
     (identity^T @ mask_slice), keeping DVE off the critical path.  exp via
     ACT with the 1/sqrt(hd) scale folded in.  ctxT[hd, q] and the softmax
     denominators (ones^T @ pT partition sums) accumulate in PSUM over key
     tiles; reciprocal + ones-outer-product broadcast + one DVE multiply
     normalize into ctxT.
  C) Output projection: out[q, :] += ctxT_h^T @ wp_h accumulated over heads.
"""
import sys
if "/opt/trn_rl_repo" not in sys.path:
    sys.path.insert(0, "/opt/trn_rl_repo")

import numpy as np

B, S, D = 2, 2048, 2048
H, HD = 16, 128
NCORES = 8
HH = 4            # heads per core
CW = HH * HD      # 512 column slice per core
P = 128
KT = D // P       # 16 contraction tiles
NQC = 4           # q chunks of 512
NKT = S // P      # 16 key tiles
SCALE = 1.0 / float(np.sqrt(HD))
MASK_NEG = -1.0e10

_cache = {}


def _build():
    import concourse.bass as bass
    import concourse.tile as tile
    from concourse import bacc, mybir

    F32 = mybir.dt.float32
    F32R = mybir.dt.float32r
    AF = mybir.ActivationFunctionType
    ALU = mybir.AluOpType

    nc = bacc.Bacc("TRN2", target_bir_lowering=False, debug=False, num_devices=NCORES)

    xt = nc.dram_tensor("xt", [D, S], F32R, kind="ExternalInput")      # x[b].T
    wq = nc.dram_tensor("wq", [D, CW], F32R, kind="ExternalInput")
    wk = nc.dram_tensor("wk", [D, CW], F32R, kind="ExternalInput")
    wv = nc.dram_tensor("wv", [D, CW], F32R, kind="ExternalInput")
    wp = nc.dram_tensor("wp", [CW, D], F32R, kind="ExternalInput")
    # bq/bk arrive host-pre-arranged as [p, h] so the load is contiguous
    bq = nc.dram_tensor("bq", [P, HH], F32, kind="ExternalInput")
    bk = nc.dram_tensor("bk", [P, HH], F32, kind="ExternalInput")
    bv = nc.dram_tensor("bv", [CW], F32, kind="ExternalInput")
    out = nc.dram_tensor("out", [S, D], F32, kind="ExternalOutput")

    with tile.TileContext(nc) as tc:
        with tc.tile_pool(name="consts", bufs=1) as consts, \
             tc.tile_pool(name="dram", bufs=1, space="DRAM") as dram:
            # DRAM scratch: qT/kT [head][hd, S], v [head][S, hd] (contiguous
            # per-head so phase B loads are linear 64KB copies)
            qT_d = dram.tile([HH, P, S], F32R)
            v_d = dram.tile([S, CW], F32R)

            # per-head per-partition biases for qT/kT layout: [p, h]
            # (SWDGE so the sync/scalar queues start on xt immediately)
            bq_sb = consts.tile([P, HH], F32)
            bk_sb = consts.tile([P, HH], F32)
            bv_sb = consts.tile([P, CW], F32)
            # ones vectors (fp32r) for denominator / broadcast matmuls
            ones_f32 = consts.tile([P, 1], F32)
            nc.vector.memset(ones_f32, 1.0)
            ones_col = consts.tile([P, 1], F32R)
            nc.vector.tensor_copy(ones_col, ones_f32)
            ones_row_f32 = consts.tile([1, P], F32)
            nc.vector.memset(ones_row_f32, 1.0)
            ones_row = consts.tile([1, P], F32R)
            nc.vector.tensor_copy(ones_row, ones_row_f32)
            # causal masks + identity built up front (values 0/-1e10/1 are
            # exact in any float width, so build straight into fp32r)
            mask_r = consts.tile([P, 896], F32R)
            nc.vector.memset(mask_r.bitcast(F32), 0.0)
            nc.gpsimd.affine_select(
                out=mask_r, in_=mask_r,
                compare_op=ALU.is_ge, fill=MASK_NEG,
                base=-384, channel_multiplier=-1, pattern=[[1, 896]],
            )
            ident_r = consts.tile([P, P], F32R)
            nc.vector.memset(ident_r.bitcast(F32), 0.0)
            nc.gpsimd.affine_select(
                out=ident_r, in_=ident_r,
                compare_op=ALU.not_equal, fill=1.0,
                base=0, channel_multiplier=1, pattern=[[-1, P]],
            )

            # kT for all heads stays in SBUF through phase B (saves the
            # DRAM round-trip and B-side reloads)
            kT_all = consts.tile([P, HH, S], F32R)

            # ---------------- Phase A: QKV projections ----------------
            with tc.tile_pool(name="xt_pool", bufs=2 * KT) as xt_pool, \
                 tc.tile_pool(name="w_pool", bufs=3 * KT) as w_pool, \
                 tc.tile_pool(name="stA", bufs=4) as stA, \
                 tc.tile_pool(name="psA", bufs=8, space="PSUM") as psA:

                HQ = [nc.sync, nc.scalar]
                # weights on SWDGE upfront; xt on the two HWDGE queues in
                # per-chunk [128, 512] tiles issued chunk-major so the first
                # q chunk's operands arrive first (the DMA fabric is a shared
                # serial resource -- JIT arrival order matters)
                w_ts = {}
                for wname, wdram in (("wq", wq), ("wk", wk)):
                    lst = []
                    for kt in range(KT):
                        t = w_pool.tile([P, CW], F32R, tag="w",
                                        name=f"{wname}_{kt}")
                        nc.gpsimd.dma_start(t, wdram[kt * P:(kt + 1) * P, :])
                        lst.append(t)
                    w_ts[wname] = lst
                    if wname == "wq":
                        nc.gpsimd.dma_start(bq_sb, bq[:])
                        nc.gpsimd.dma_start(bk_sb, bk[:])
                xt_t = [[None] * NQC for _ in range(KT)]

                def load_xt_chunk(c4):
                    for kt in range(KT):
                        t = xt_pool.tile([P, 512], F32R, tag="xt",
                                         name=f"xt{kt}_{c4}")
                        HQ[kt % 2].dma_start(
                            t, xt[kt * P:(kt + 1) * P, c4 * 512:(c4 + 1) * 512])
                        xt_t[kt][c4] = t

                load_xt_chunk(0)
                load_xt_chunk(1)
                # wv + bv on the scalar HWDGE queue: lands after the first two
                # xt chunks, before the first v sub-pass needs it (SWDGE
                # descriptor-gen would deliver it too late)
                lst = []
                for kt in range(KT):
                    t = w_pool.tile([P, CW], F32R, tag="w", name=f"wv_{kt}")
                    nc.scalar.dma_start(t, wv[kt * P:(kt + 1) * P, :])
                    lst.append(t)
                w_ts["wv"] = lst
                nc.scalar.dma_start(
                    bv_sb, bass.AP(tensor=bv, offset=0, ap=[[0, P], [1, CW]])
                )
                load_xt_chunk(2)
                load_xt_chunk(3)

                # PE warm-up: spin matmuls on the const tiles while the
                # first xt/wq DMAs are in flight -- keeps the HAM clock-gate
                # warm so the first real matmuls run at full rate
                ps_warm = psA.tile([P, 512], F32, tag="psA", name="ps_warm")
                for wi in range(14):
                    nc.tensor.matmul(ps_warm, ident_r, mask_r[:, 128:640],
                                     start=True, stop=True)

                def a_qk(wname, c4):
                    bias_sb = bq_sb if wname == "wq" else bk_sb
                    scratch = qT_d if wname == "wq" else None
                    w_t = w_ts[wname]
                    pss = [psA.tile([P, 512], F32, tag="psA",
                                    name=f"psA{c4}_{h}") for h in range(HH)]
                    for kt in range(KT):
                        for h in range(HH):
                            nc.tensor.matmul(
                                pss[h],
                                w_t[kt][:, h * HD:(h + 1) * HD],
                                xt_t[kt][c4],
                                start=(kt == 0), stop=(kt == KT - 1),
                            )
                    for h in range(HH):
                        if scratch is None:
                            # kT: bias-add straight into resident SBUF
                            nc.scalar.activation(
                                kT_all[:, h, c4 * 512:(c4 + 1) * 512],
                                pss[h], AF.Identity,
                                bias=bias_sb[:, h:h + 1], scale=1.0,
                            )
                        else:
                            st = stA.tile([P, 512], F32R, tag="stA",
                                          name="stA_qk")
                            nc.scalar.activation(
                                st, pss[h], AF.Identity,
                                bias=bias_sb[:, h:h + 1], scale=1.0,
                            )
                            nc.gpsimd.dma_start(
                                scratch[h][:, c4 * 512:(c4 + 1) * 512], st
                            )

                def a_v(c4):
                    w_t = w_ts["wv"]
                    for st16 in range(4 * c4, 4 * c4 + 4):
                        psv = psA.tile([P, 512], F32, tag="psA",
                                       name=f"psV{st16}")
                        for kt in range(KT):
                            nc.tensor.matmul(
                                psv,
                                xt_t[kt][c4][:, (st16 % 4) * P:
                                             (st16 % 4 + 1) * P],
                                w_t[kt],
                                start=(kt == 0), stop=(kt == KT - 1),
                            )
                        st = stA.tile([P, 512], F32R, tag="stA", name="stA_v")
                        nc.vector.tensor_tensor(st, psv, bv_sb, ALU.add)
                        HQ[st16 % 2].dma_start(
                            v_d[st16 * P:(st16 + 1) * P, :], st)

                # chunk-group order: xt chunk c4 dies after a_v(c4), so only
                # two chunks of xt are ever resident
                a_qk("wq", 0)
                a_qk("wq", 1)
                a_qk("wk", 0)
                a_v(0)
                a_qk("wq", 2)
                a_qk("wk", 1)
                a_v(1)
                a_qk("wq", 3)
                a_qk("wk", 2)
                a_v(2)
                a_qk("wk", 3)
                a_v(3)

            # ---------------- Phases B+C shared tiles ----------------
            with tc.tile_pool(name="bc_pool", bufs=1) as bc_pool:
                ctxT_sb = bc_pool.tile([P, HH, S], F32R)

                # ---------------- Phase B: attention ----------------
                with tc.tile_pool(name="qkv_pool", bufs=2) as qkv_pool, \
                     tc.tile_pool(name="vh_pool", bufs=2) as vh_pool, \
                     tc.tile_pool(name="pT_pool", bufs=2 * NKT + 12) as pT_pool, \
                     tc.tile_pool(name="accB", bufs=2) as accB, \
                     tc.tile_pool(name="stB", bufs=2) as stB, \
                     tc.tile_pool(name="psS", bufs=4, space="PSUM") as psS, \
                     tc.tile_pool(name="psCtx", bufs=3, space="PSUM") as psCtx, \
                     tc.tile_pool(name="psT", bufs=1, space="PSUM") as psT, \
                     tc.tile_pool(name="ppool", bufs=4) as ppool, \
                     nc.allow_low_precision(
                         reason="float32r tiles are 4-byte fp32 containers; "
                                "PE rounds on read, DVE writes full fp32 bits"):

                    def b_scores(h, j, qT_sb):
                        # scoresT blocks + exp for q chunk j; diagonal blocks
                        # get the causal mask added in PSUM by a 2nd matmul
                        nkt = 4 * j + 4
                        qs = qT_sb[:, j * 512:(j + 1) * 512]
                        pt_t = []
                        for i in range(nkt):
                            ps_s = psS.tile([P, 512], F32, tag="ps_s")
                            m = i - 4 * j
                            # diagonal blocks: only columns >= 128*m are live;
                            # strip the matmul when the narrower width still
                            # runs at 1 cyc/row (fp32r needs N >= 256)
                            c0 = P * m if m in (1, 2) else 0
                            nc.tensor.matmul(
                                ps_s[:, c0:],
                                kT_all[:, h, i * P:(i + 1) * P], qs[:, c0:],
                                start=True, stop=(m < 0),
                            )
                            if m >= 0:
                                nc.tensor.matmul(
                                    ps_s[:, c0:], ident_r,
                                    mask_r[:, 384 - P * m + c0:896 - P * m],
                                    start=False, stop=True,
                                )
                            pt = pT_pool.tile([P, 512], F32R, tag="pt",
                                              name=f"pt{h}_{j}_{i}")
                            if m > 0:
                                # columns < 128*m are fully masked: zero them
                                # on DVE and exp only the live strip (ACT is
                                # the phase-B pacer)
                                nc.vector.memset(
                                    pt.bitcast(F32)[:, :P * m], 0.0)
                                nc.scalar.activation(
                                    pt[:, P * m:], ps_s[:, P * m:],
                                    AF.Exp, scale=SCALE)
                            else:
                                nc.scalar.activation(pt, ps_s, AF.Exp,
                                                     scale=SCALE)
                            pt_t.append(pt)
                        return pt_t

                    def b_tail(h, j, v_t, pt_t):
                        # ctxT and denominator PSUM accumulations, then
                        # normalize into ctxT_sb
                        nkt = 4 * j + 4
                        ps_c = psCtx.tile([P, 512], F32, tag="ps_c")
                        for i in range(nkt):
                            m = i - 4 * j
                            c0 = P * m if m in (1, 2) else 0
                            nc.tensor.matmul(
                                ps_c[:, c0:], v_t[i], pt_t[i][:, c0:],
                                start=(i == 0), stop=(i == nkt - 1),
                            )
                        # pairwise pre-sums on DVE halve the denominator
                        # matmul count
                        npair = nkt // 2
                        psums = []
                        for i in range(npair):
                            pp = ppool.tile([P, 512], F32R, tag="ppair",
                                            name=f"pp{h}_{j}_{i}")
                            nc.vector.tensor_tensor(
                                pp, pt_t[2 * i], pt_t[2 * i + 1], ALU.add)
                            psums.append(pp)
                        ps_d = psT.tile([1, 512], F32, tag="ps_db", name="ps_d")
                        for i in range(npair):
                            nc.tensor.matmul(
                                ps_d, ones_col, psums[i],
                                start=(i == 0), stop=(i == npair - 1),
                            )
                        rden = accB.tile([1, 512], F32R, tag="rden")
                        nc.vector.reciprocal(rden, ps_d)
                        ps_b = psT.tile([P, 512], F32, tag="ps_db", name="ps_b")
                        nc.tensor.matmul(ps_b, ones_row, rden,
                                         start=True, stop=True)
                        rdenb = stB.tile([P, 512], F32, tag="rdenb")
                        nc.vector.tensor_copy(rdenb, ps_b)
                        nc.vector.tensor_tensor(
                            ctxT_sb[:, h, j * 512:(j + 1) * 512],
                            ps_c, rdenb, ALU.mult,
                        )

                    HQ = [nc.sync, nc.scalar]

                    def load_head(h):
                        qT_sb = qkv_pool.tile([P, S], F32R, tag="qT",
                                              name=f"qT{h}")
                        nc.sync.dma_start(qT_sb, qT_d[h])
                        # one rearranged 1MB DMA instead of 16 per-tile loads:
                        # HWDGE queue-processing time (~0.6us per dma) was
                        # stalling the seam, not bandwidth
                        v_all = vh_pool.tile([P, NKT, HD], F32R, tag="vh",
                                             name=f"vh{h}")
                        HQ[h % 2].dma_start(
                            v_all,
                            v_d[:, h * HD:(h + 1) * HD].rearrange(
                                "(i p) d -> p i d", p=P),
                        )
                        v_t = [v_all[:, i, :] for i in range(NKT)]
                        return qT_sb, v_t

                    # tails lag scores by two chunks: the ACT exp stream of
                    # chunk j must finish before tail(j)'s last ctx matmul,
                    # so give PE two chunks of score work to chew in between
                    from collections import deque
                    pend = deque()
                    loaded = load_head(0)
                    for h in range(HH):
                        qT_sb, v_t = loaded
                        if h + 1 < HH:
                            loaded = load_head(h + 1)
                        for j in range(NQC):
                            pt_t = b_scores(h, j, qT_sb)
                            pend.append((h, j, v_t, pt_t))
                            # at a head boundary the j=3 tail needs 16 ACT
                            # exps; delay it one extra score block so the PE
                            # has work while ACT drains
                            if j == NQC - 1 and h < HH - 1:
                                continue
                            if len(pend) > 1:
                                b_tail(*pend.popleft())
                    while pend:
                        b_tail(*pend.popleft())

                # ---------------- Phase C: output projection ----------------
                with tc.tile_pool(name="wp_pool", bufs=HH) as wp_pool, \
                     tc.tile_pool(name="outC", bufs=8) as outC, \
                     tc.tile_pool(name="psC", bufs=8, space="PSUM") as psC:
                    wp_t = []
                    for hh in range(HH):
                        t = wp_pool.tile([P, D], F32R, tag="wp", name=f"wp{hh}")
                        nc.gpsimd.dma_start(t, wp[hh * P:(hh + 1) * P, :])
                        wp_t.append(t)
                    for t16 in range(NKT):
                        for c4 in range(NQC):
                            ps_o = psC.tile([P, 512], F32, tag="psC",
                                            name=f"psC{t16}_{c4}")
                            for hh in range(HH):
                                nc.tensor.matmul(
                                    ps_o,
                                    ctxT_sb[:, hh, t16 * P:(t16 + 1) * P],
                                    wp_t[hh][:, c4 * 512:(c4 + 1) * 512],
                                    start=(hh == 0), stop=(hh == HH - 1),
                                )
                            o_st = outC.tile([P, 512], F32, tag="out",
                                             name=f"out{t16}_{c4}")
                            nc.any.tensor_copy(o_st, ps_o)
                            [nc.sync, nc.scalar][(t16 + c4) % 2].dma_start(
                                out[t16 * P:(t16 + 1) * P,
                                    c4 * 512:(c4 + 1) * 512], o_st)

    nc.compile()
    return nc


def _get_nc():
    if "nc" not in _cache:
        _cache["nc"] = _build()
    return _cache["nc"]


def _in_maps(x, wq, bq, wk, bk, wv, bv, wp):
    x = np.asarray(x, dtype=np.float32)
    maps = []
    xT = [np.ascontiguousarray(x[b].T) for b in range(B)]
    for c in range(NCORES):
        b = c // 4
        cols = slice((c % 4) * CW, (c % 4) * CW + CW)
        maps.append({
            "xt": xT[b],
            "wq": np.ascontiguousarray(np.asarray(wq, np.float32)[:, cols]),
            "wk": np.ascontiguousarray(np.asarray(wk, np.float32)[:, cols]),
            "wv": np.ascontiguousarray(np.asarray(wv, np.float32)[:, cols]),
            "wp": np.ascontiguousarray(np.asarray(wp, np.float32)[cols, :]),
            "bq": np.ascontiguousarray(
                np.asarray(bq, np.float32)[cols].reshape(HH, P).T),
            "bk": np.ascontiguousarray(
                np.asarray(bk, np.float32)[cols].reshape(HH, P).T),
            "bv": np.ascontiguousarray(np.asarray(bv, np.float32)[cols]),
        })
    return maps


def kernel(x, wq, bq, wk, bk, wv, bv, wp, bp):
    from concourse.bass_utils import run_bass_kernel_spmd

    nc = _get_nc()
    maps = _in_maps(x, wq, bq, wk, bk, wv, bv, wp)
    res = run_bass_kernel_spmd(nc, maps, core_ids=list(range(NCORES)))
    parts = [res.results[c]["out"] for c in range(NCORES)]
    bp = np.asarray(bp, dtype=np.float32)
    full = np.empty((B, S, D), dtype=np.float32)
    for b in range(B):
        acc = parts[4 * b].astype(np.float64)
        for c in range(4 * b + 1, 4 * b + 4):
            acc += parts[c]
        full[b] = (acc + bp).astype(np.float32)
    return full



# revision 36
# speedup vs baseline: 1.1997x; 1.1997x over previous
"""Multi-head causal attention (B=2, S=2048, D=2048, H=16) on 8 TRN2 NeuronCores.

Sharding (host-side): core c in 0..7 handles batch b=c//4 and heads
4*(c%4)..4*(c%4)+4 (a 512-wide column slice of wq/wk/wv, row slice of wp).
Each core computes its 4 heads' attention and a partial output projection
[S, D]; the host sums the 4 partials per batch and adds bp.

Per-core kernel (all matmuls in float32r -> full PE speed, ~2e-4 rel err):
  A) QKV projections from host-pre-transposed xT (d-major):
       qT/kT per head in [hd=128, S] layout, v per head in [S, hd] natural
       layout, written to DRAM scratch.
  B) Per head, per 512-wide q chunk j: scoresT = K_tile @ Q_chunk in
     [keys, q] layout (causal: only key tiles <= diagonal).  The causal mask
     for diagonal blocks is ADDED IN PSUM by a second matmul
     (identity^T @ mask_slice), keeping DVE off the critical path.  exp via
     ACT with the 1/sqrt(hd) scale folded in.  ctxT[hd, q] and the softmax
     denominators (ones^T @ pT partition sums) accumulate in PSUM over key
     tiles; reciprocal + ones-outer-product broadcast + one DVE multiply
     normalize into ctxT.
  C) Output projection: out[q, :] += ctxT_h^T @ wp_h accumulated over heads.
"""
import sys
if "/opt/trn_rl_repo" not in sys.path:
    sys.path.insert(0, "/opt/trn_rl_repo")

import numpy as np

B, S, D = 2, 2048, 2048
H, HD = 16, 128
NCORES = 8
HH = 4            # heads per core
CW = HH * HD      # 512 column slice per core
P = 128
KT = D // P       # 16 contraction tiles
NQC = 4           # q chunks of 512
NKT = S // P      # 16 key tiles
SCALE = 1.0 / float(np.sqrt(HD))
MASK_NEG = -1.0e10

_cache = {}


def _build():
    import concourse.bass as bass
    import concourse.tile as tile
    from concourse import bacc, mybir

    F32 = mybir.dt.float32
    F16 = mybir.dt.float16
    F32R = mybir.dt.float32r
    AF = mybir.ActivationFunctionType
    ALU = mybir.AluOpType

    FP8 = mybir.dt.float8e4
    DR = mybir.MatmulPerfMode.DoubleRow

    nc = bacc.Bacc("TRN2", target_bir_lowering=False, debug=False, num_devices=NCORES)

    # x and the projection weights arrive host-split into fp8e4 hi+lo pairs
    # (w pre-scaled by 32 so its mass clears the e4m3 subnormal floor; the
    # 1/32 is folded into the psum-readout scale).  Layouts are PE-ready:
    # x [chunk][p][kt][512 tokens], w [p][kt][out-col].
    xh = nc.dram_tensor("xh", [NQC, P, KT, 512], FP8, kind="ExternalInput")
    xl = nc.dram_tensor("xl", [NQC, P, KT, 512], FP8, kind="ExternalInput")
    w8 = {}
    for wn in ("wq", "wk", "wv"):
        for part in ("h", "l"):
            w8[wn + part] = nc.dram_tensor(
                wn + part, [P, KT, CW], FP8, kind="ExternalInput")
    wph = nc.dram_tensor("wph", [P, HH, D], FP8, kind="ExternalInput")
    wpl = nc.dram_tensor("wpl", [P, HH, D], FP8, kind="ExternalInput")
    # bq/bk arrive host-pre-arranged as [p, h] so the load is contiguous
    bq = nc.dram_tensor("bq", [P, HH], F32, kind="ExternalInput")
    bk = nc.dram_tensor("bk", [P, HH], F32, kind="ExternalInput")
    bv = nc.dram_tensor("bv", [CW], F32, kind="ExternalInput")   # x32 on host
    out = nc.dram_tensor("out", [S, D], F16, kind="ExternalOutput")

    with tile.TileContext(nc) as tc:
        with tc.tile_pool(name="consts", bufs=1) as consts, \
             tc.tile_pool(name="dram", bufs=1, space="DRAM") as dram:
            # DRAM scratch: qT/kT [head][hd, S], v [head][S, hd] (contiguous
            # per-head so phase B loads are linear 64KB copies)
            qT_d = dram.tile([HH, P, S], F32R)
            v_d = dram.tile([S, CW], F32R)

            # per-head per-partition biases for qT/kT layout: [p, h]
            # (SWDGE so the sync/scalar queues start on xt immediately)
            bq_sb = consts.tile([P, HH], F32)
            bk_sb = consts.tile([P, HH], F32)
            # denominator contraction vector: value 32 so the reciprocal
            # also undoes the 32x scale carried by v (wv was host-scaled)
            ones_f32 = consts.tile([P, 1], F32)
            nc.vector.memset(ones_f32, 32.0)
            ones_col = consts.tile([P, 1], F32R)
            nc.vector.tensor_copy(ones_col, ones_f32)
            ident_r = consts.tile([P, P], F32R)
            nc.vector.memset(ident_r.bitcast(F32), 0.0)
            nc.gpsimd.affine_select(
                out=ident_r, in_=ident_r,
                compare_op=ALU.not_equal, fill=1.0,
                base=0, channel_multiplier=1, pattern=[[-1, P]],
            )

            # kT for all heads stays in SBUF through phase B (saves the
            # DRAM round-trip and B-side reloads)
            kT_all = consts.tile([P, HH, S], F32R)

            # ---------------- Phase A: QKV projections ----------------
            # fp8 DoubleRow: each matmul contracts TWO 128-deep k-tiles at
            # 0.5 cyc/row -> 4x fp32r throughput; the hi/lo error split costs
            # 3 terms (hi*hi + hi*lo + lo*hi) for a net 0.75x PE time
            with tc.tile_pool(name="xt_pool", bufs=4) as xt_pool, \
                 tc.tile_pool(name="w_pool", bufs=6) as w_pool, \
                 tc.tile_pool(name="stA", bufs=4) as stA, \
                 tc.tile_pool(name="aconsts", bufs=1) as aconsts, \
                 tc.tile_pool(name="psA", bufs=8, space="PSUM") as psA:

                HQ = [nc.sync, nc.scalar]
                bv_sb = aconsts.tile([P, CW], F32)
                # warm-up operand (phase-A scope only)
                warm_r = aconsts.tile([P, 256], F32R)
                nc.vector.memset(warm_r.bitcast(F32), 0.0)

                # weights on SWDGE in kt-halves (region DMAs into one tile
                # per tensor) so the first matmuls start before full arrival;
                # xt chunks stream on the two HWDGE queues in kt-quarters
                w_ts = {}
                for wname in ("wqh", "wql", "wkh", "wkl"):
                    t = w_pool.tile([P, KT, CW], FP8, tag="w", name=wname)
                    nc.gpsimd.dma_start(t[:, 0:8, :], w8[wname][:, 0:8, :])
                    nc.gpsimd.dma_start(t[:, 8:16, :], w8[wname][:, 8:16, :])
                    w_ts[wname] = t
                    if wname == "wqh":
                        nc.gpsimd.dma_start(bq_sb, bq[:])
                        nc.gpsimd.dma_start(bk_sb, bk[:])
                xt_t = {}

                def load_xt_chunk(c4):
                    for part, src, q in (("h", xh, nc.sync),
                                         ("l", xl, nc.scalar)):
                        t = xt_pool.tile([P, KT, 512], FP8, tag="xt",
                                         name=f"xt{part}_{c4}")
                        for g in range(4):
                            q.dma_start(t[:, 4 * g:4 * g + 4, :],
                                        src[c4][:, 4 * g:4 * g + 4, :])
                        xt_t[(part, c4)] = t

                load_xt_chunk(0)
                load_xt_chunk(1)
                # wv + bv on the scalar HWDGE queue: lands after the first two
                # xt chunks, before the first v sub-pass needs it
                for wname in ("wvh", "wvl"):
                    t = w_pool.tile([P, KT, CW], FP8, tag="w", name=wname)
                    nc.scalar.dma_start(t[:, 0:8, :], w8[wname][:, 0:8, :])
                    nc.scalar.dma_start(t[:, 8:16, :], w8[wname][:, 8:16, :])
                    w_ts[wname] = t
                nc.scalar.dma_start(
                    bv_sb, bass.AP(tensor=bv, offset=0, ap=[[0, P], [1, CW]])
                )
                load_xt_chunk(2)
                load_xt_chunk(3)

                # PE warm-up: spin matmuls on the const tiles while the
                # first xt/wq DMAs are in flight -- keeps the HAM clock-gate
                # warm so the first real matmuls run at full rate
                ps_warm = psA.tile([P, 256], F32, tag="psA", name="ps_warm")
                for wi in range(14):
                    nc.tensor.matmul(ps_warm, ident_r, warm_r,
                                     start=True, stop=True)

                def dr_terms(wname, c4):
                    # (lhs_tile, rhs_tile) per compensation term
                    return ((w_ts[wname + "h"], xt_t[("h", c4)]),
                            (w_ts[wname + "h"], xt_t[("l", c4)]),
                            (w_ts[wname + "l"], xt_t[("h", c4)]))

                def a_qk(wname, c4):
                    bias_sb = bq_sb if wname == "wq" else bk_sb
                    scratch = qT_d if wname == "wq" else None
                    pss = [psA.tile([P, 512], F32, tag="psA",
                                    name=f"psA{c4}_{h}") for h in range(HH)]
                    terms = dr_terms(wname, c4)
                    # n0-outer so each tile has one pending psum group at
                    # a time; heads stay interleaved so matmuls chase the
                    # DMA arrival order tile-by-tile
                    for n0 in (0, 256):
                        for ti, (wt, xt8) in enumerate(terms):
                            for t in range(KT // 2):
                                for h in range(HH):
                                    nc.tensor.matmul(
                                        pss[h][:, n0:n0 + 256],
                                        wt[:, 2 * t:2 * t + 2,
                                           h * HD:(h + 1) * HD],
                                        xt8[:, 2 * t:2 * t + 2,
                                            n0:n0 + 256],
                                        start=(ti == 0 and t == 0),
                                        stop=(ti == 2 and t == KT // 2 - 1),
                                        perf_mode=DR,
                                    )
                    for h in range(HH):
                        if scratch is None:
                            # kT: bias-add + 1/32 w-scale undo, straight into
                            # resident SBUF
                            nc.scalar.activation(
                                kT_all[:, h, c4 * 512:(c4 + 1) * 512],
                                pss[h], AF.Identity,
                                bias=bias_sb[:, h:h + 1], scale=1.0 / 32.0,
                            )
                        else:
                            st = stA.tile([P, 512], F32R, tag="stA",
                                          name="stA_qk")
                            nc.scalar.activation(
                                st, pss[h], AF.Identity,
                                bias=bias_sb[:, h:h + 1], scale=1.0 / 32.0,
                            )
                            nc.gpsimd.dma_start(
                                scratch[h][:, c4 * 512:(c4 + 1) * 512], st
                            )

                def a_v(c4):
                    terms = dr_terms("wv", c4)
                    psvs = [psA.tile([P, 512], F32, tag="psA",
                                     name=f"psV{4 * c4 + s}")
                            for s in range(4)]
                    for n0 in (0, 256):
                        for ti, (wt, xt8) in enumerate(terms):
                            for t in range(KT // 2):
                                for s in range(4):
                                    nc.tensor.matmul(
                                        psvs[s][:, n0:n0 + 256],
                                        xt8[:, 2 * t:2 * t + 2,
                                            s * P:(s + 1) * P],
                                        wt[:, 2 * t:2 * t + 2, n0:n0 + 256],
                                        start=(ti == 0 and t == 0),
                                        stop=(ti == 2 and t == KT // 2 - 1),
                                        perf_mode=DR,
                                    )
                    for s in range(4):
                        st16 = 4 * c4 + s
                        # v stays 32x-scaled (bv host-scaled to match); the
                        # denominator reciprocal undoes it via ones_col=32
                        st = stA.tile([P, 512], F32R, tag="stA", name="stA_v")
                        nc.vector.tensor_tensor(st, psvs[s], bv_sb, ALU.add)
                        HQ[st16 % 2].dma_start(
                            v_d[st16 * P:(st16 + 1) * P, :], st)

                # chunk-group order: xt chunk c4 dies after a_v(c4), so only
                # two chunks of xt are ever resident
                a_qk("wq", 0)
                a_qk("wq", 1)
                a_qk("wk", 0)
                a_v(0)
                a_qk("wq", 2)
                a_qk("wk", 1)
                a_v(1)
                a_qk("wq", 3)
                a_qk("wk", 2)
                a_v(2)
                a_qk("wk", 3)
                a_v(3)

            # ---------------- Phases B+C shared tiles ----------------
            with tc.tile_pool(name="bc_pool", bufs=1) as bc_pool, \
                 tc.tile_pool(name="wp_pool", bufs=HH) as wp_pool:
                # ctx kept as an fp8e4 hi/lo pair (natural scale) for
                # phase C's DoubleRow matmuls; the f32 value only exists in a
                # small per-chunk scratch
                cxh = bc_pool.tile([P, HH, S], FP8)
                cxl = bc_pool.tile([P, HH, S], FP8)
                wp_t = []

                # ---------------- Phase B: attention ----------------
                # per-chunk denominator split knobs: DVE pairwise pre-sums,
                # then Pool quad pre-sums, remainder via ones-matmuls on PE
                NQUAD = {0: 0, 1: 0, 2: 0, 3: 0}
                with tc.tile_pool(name="qkv_pool", bufs=2) as qkv_pool, \
                     tc.tile_pool(name="vh_pool", bufs=2) as vh_pool, \
                     tc.tile_pool(name="pT_pool", bufs=33) as pT_pool, \
                     tc.tile_pool(name="accB", bufs=1) as accB, \
                     tc.tile_pool(name="stB", bufs=1) as stB, \
                     tc.tile_pool(name="psS", bufs=4, space="PSUM") as psS, \
                     tc.tile_pool(name="psCtx", bufs=3, space="PSUM") as psCtx, \
                     tc.tile_pool(name="psT", bufs=1, space="PSUM") as psT, \
                     tc.tile_pool(name="ppool", bufs=3) as ppool, \
                     tc.tile_pool(name="qpool", bufs=1) as qpool, \
                     tc.tile_pool(name="cpool", bufs=2) as cpool, \
                     nc.allow_low_precision(
                         reason="float32r tiles are 4-byte fp32 containers; "
                                "PE rounds on read, DVE writes full fp32 bits"):

                    # matmul strip starts per diagonal sub-tile (N >= 256
                    # keeps fp32r at 1 cyc/row, so m=3 computes from 256)
                    C0_MM = {0: 0, 1: P, 2: 2 * P, 3: 2 * P}

                    def b_scores(h, j, qT_sb):
                        # scoresT blocks + exp for q chunk j; causal masking
                        # of diagonal blocks: Pool memsets the dead strip and
                        # affine_select zeros the sub-diagonal corner post-exp
                        nkt = 4 * j + 4
                        qs = qT_sb[:, j * 512:(j + 1) * 512]
                        pt_t = []
                        for i in range(nkt):
                            ps_s = psS.tile([P, 512], F32, tag="ps_s")
                            m = i - 4 * j
                            c0 = C0_MM[m] if m >= 0 else 0
                            nc.tensor.matmul(
                                ps_s[:, c0:],
                                kT_all[:, h, i * P:(i + 1) * P], qs[:, c0:],
                                start=True, stop=True,
                            )
                            pt = pT_pool.tile([P, 512], F32R, tag="pt",
                                              name=f"pt{h}_{j}_{i}")
                            if m > 0:
                                nc.gpsimd.memset(
                                    pt.bitcast(F32)[:, :P * m], 0.0)
                                nc.scalar.activation(
                                    pt[:, P * m:], ps_s[:, P * m:],
                                    AF.Exp, scale=SCALE)
                            else:
                                nc.scalar.activation(pt, ps_s, AF.Exp,
                                                     scale=SCALE)
                            if m >= 0:
                                # zero the strictly-sub-diagonal corner
                                # (query col < key row)
                                nc.gpsimd.affine_select(
                                    out=pt[:, P * m:P * (m + 1)],
                                    in_=pt[:, P * m:P * (m + 1)],
                                    compare_op=ALU.is_ge, fill=0.0,
                                    base=0, channel_multiplier=-1,
                                    pattern=[[1, P]],
                                )
                            pt_t.append(pt)
                        return pt_t

                    def b_tail(h, j, v_t, pt_t):
                        # ctxT and denominator PSUM accumulations, then
                        # normalize into ctxT_sb
                        nkt = 4 * j + 4
                        ps_c = psCtx.tile([P, 512], F32, tag="ps_c")
                        if j == 0:
                            # order m0, m1, m2, m3(full): m0 starts the full
                            # region, m3 full-width carries the stop (its
                            # masked cols read zeros from pt)
                            order = [(0, 0, True, False), (1, P, False, False),
                                     (2, 2 * P, False, False),
                                     (3, 0, False, True)]
                            for m, c0, st, sp in order:
                                nc.tensor.matmul(
                                    ps_c[:, c0:], v_t[m], pt_t[m][:, c0:],
                                    start=st, stop=sp,
                                )
                        else:
                            # off-diagonals first (i=0 starts full region),
                            # then m1/m2/m3 narrowed, m0 last carries stop
                            for i in range(4 * j):
                                nc.tensor.matmul(
                                    ps_c, v_t[i], pt_t[i],
                                    start=(i == 0), stop=False,
                                )
                            for m in (1, 2, 3):
                                c0 = C0_MM[m]
                                nc.tensor.matmul(
                                    ps_c[:, c0:], v_t[4 * j + m],
                                    pt_t[4 * j + m][:, c0:],
                                    start=False, stop=False,
                                )
                            nc.tensor.matmul(
                                ps_c, v_t[4 * j], pt_t[4 * j],
                                start=False, stop=True,
                            )
        # denominator: DVE pairwise pre-sums, Pool quads for the
                        # leading pairs, ones-matmuls on PE contract the
                        # stream into ps_d; each matmul is issued as soon as
                        # its operand exists so the small pools recycle
                        npair = nkt // 2
                        nquad = NQUAD[j]
                        nmm = nquad + (npair - 2 * nquad)
                        ps_d = psT.tile([1, 512], F32, tag="ps_db", name="ps_d")
                        k = 0
                        prev = None
                        for i in range(npair):
                            pp = ppool.tile([P, 512], F32R, tag="ppair",
                                            name=f"pp{h}_{j}_{i}")
                            nc.vector.tensor_tensor(
                                pp, pt_t[2 * i], pt_t[2 * i + 1], ALU.add)
                            if i // 2 < nquad:
                                if i % 2 == 0:
                                    prev = pp
                                    continue
                                src = qpool.tile([P, 512], F32R, tag="quad",
                                                 name=f"qq{h}_{j}_{i}")
                                nc.gpsimd.tensor_tensor(
                                    src, prev, pp, ALU.add)
                            else:
                                src = pp
                            nc.tensor.matmul(
                                ps_d, ones_col, src,
                                start=(k == 0), stop=(k == nmm - 1),
                            )
                            k += 1
                        rden = accB.tile([1, 512], F32, tag="rden")
                        nc.vector.reciprocal(rden, ps_d)
                        rdenb = stB.tile([P, 512], F32, tag="rdenb")
                        nc.gpsimd.partition_broadcast(rdenb, rden)
                        jsl = slice(j * 512, (j + 1) * 512)
                        ct = cpool.tile([P, 512], F32, tag="ct",
                                        name=f"ct{h}_{j}")
                        nc.vector.tensor_tensor(ct, ps_c, rdenb, ALU.mult)
                        nc.gpsimd.tensor_copy(cxh[:, h, jsl], ct)
                        nc.vector.tensor_tensor(
                            cxl[:, h, jsl], ct, cxh[:, h, jsl], ALU.subtract)

                    HQ = [nc.sync, nc.scalar]

                    def load_head(h):
                        qT_sb = qkv_pool.tile([P, S], F32R, tag="qT",
                                              name=f"qT{h}")
                        # per-chunk pieces: piece c4 only depends on phase A's
                        # a_qk("wq", c4) pass, so head 0 starts early
                        for c4 in range(NQC):
                            nc.sync.dma_start(
                                qT_sb[:, c4 * 512:(c4 + 1) * 512],
                                qT_d[h][:, c4 * 512:(c4 + 1) * 512])
                        # one rearranged 1MB DMA instead of 16 per-tile loads:
                        # HWDGE queue-processing time (~0.6us per dma) was
                        # stalling the seam, not bandwidth
                        v_all = vh_pool.tile([P, NKT, HD], F32R, tag="vh",
                                             name=f"vh{h}")
                        # head 0 in 4 row-group pieces (piece g only waits on
                        # a_v(g), so phase B starts before a_v(3) lands);
                        # later heads' v_d is long since complete
                        groups = range(4) if h == 0 else (None,)
                        for g in groups:
                            sl = (slice(0, S) if g is None
                                  else slice(g * 512, (g + 1) * 512))
                            dsl = (slice(0, NKT) if g is None
                                   else slice(4 * g, 4 * g + 4))
                            HQ[h % 2].dma_start(
                                v_all[:, dsl, :],
                                v_d[sl, h * HD:(h + 1) * HD].rearrange(
                                    "(i p) d -> p i d", p=P),
                            )
                        v_t = [v_all[:, i, :] for i in range(NKT)]
                        return qT_sb, v_t

                    # tails lag scores by two chunks: the ACT exp stream of
                    # chunk j must finish before tail(j)'s last ctx matmul,
                    # so give PE two chunks of score work to chew in between
                    from collections import deque
                    pend = deque()
                    loaded = load_head(0)
                    for h in range(HH):
                        qT_sb, v_t = loaded
                        if h + 1 < HH:
                            loaded = load_head(h + 1)
                        if h == 1:
                            # wp prefetch for phase C: issued once head 0/1
                            # loads are queued so it never delays them on the
                            # shared DMA engines
                            for i, wsrc in enumerate((wph, wpl)):
                                t = wp_pool.tile([P, HH, D], FP8, tag="wp",
                                                 name=f"wp8_{i}")
                                HQ[i % 2].dma_start(t, wsrc[:])
                                wp_t.append(t)
                        for j in range(NQC):
                            pt_t = b_scores(h, j, qT_sb)
                            pend.append((h, j, v_t, pt_t))
                            # at a head boundary the j=3 tail needs 16 ACT
                            # exps; delay it one extra score block so the PE
                            # has work while ACT drains
                            if j == NQC - 1 and h < HH - 1:
                                continue
                            if len(pend) > 1:
                                b_tail(*pend.popleft())
                    while pend:
                        b_tail(*pend.popleft())

                # ---------------- Phase C: output projection ----------------
                # fp8 DoubleRow over hh-pairs, 3 hi/lo terms; ACT undoes the
                # 32x wp scale on psum readout and emits fp16 (the host sums
                # partials in float64 anyway)
                with tc.tile_pool(name="outC", bufs=8) as outC, \
                     tc.tile_pool(name="psC", bufs=8, space="PSUM") as psC:
                    cterms = ((cxh, wp_t[0]), (cxh, wp_t[1]), (cxl, wp_t[0]))
                    for t16 in range(NKT):
                        for c4 in range(NQC):
                            ps_o = psC.tile([P, 512], F32, tag="psC",
                                            name=f"psC{t16}_{c4}")
                            for n0 in (0, 256):
                                for ti, (cx, wpt) in enumerate(cterms):
                                    for g in range(HH // 2):
                                        nc.tensor.matmul(
                                            ps_o[:, n0:n0 + 256],
                                            cx[:, 2 * g:2 * g + 2,
                                               t16 * P:(t16 + 1) * P],
                                            wpt[:, 2 * g:2 * g + 2,
                                                c4 * 512 + n0:
                                                c4 * 512 + n0 + 256],
                                            start=(ti == 0 and g == 0),
                                            stop=(ti == 2 and
                                                  g == HH // 2 - 1),
                                            perf_mode=DR,
                                        )
                            o_st = outC.tile([P, 512], F16, tag="out",
                                             name=f"out{t16}_{c4}")
                            # GPSIMD cannot read PSUM on hardware: the
                            # scale-and-convert copies alternate ACT/DVE only
                            if (t16 + c4) % 2 == 0:
                                nc.scalar.activation(o_st, ps_o, AF.Identity,
                                                     scale=1.0 / 32.0)
                            else:
                                nc.vector.tensor_scalar(
                                    o_st, ps_o, 1.0 / 32.0, None, ALU.mult)
                            [nc.sync, nc.scalar][(t16 + c4) % 2].dma_start(
                                out[t16 * P:(t16 + 1) * P,
                                    c4 * 512:(c4 + 1) * 512], o_st)

    nc.compile()
    return nc


def _get_nc():
    if "nc" not in _cache:
        _cache["nc"] = _build()
    return _cache["nc"]


def _split8(a):
    """fp8e4 hi/lo error split: a ~= hi + lo with ~0.13% residual."""
    import ml_dtypes
    E4 = ml_dtypes.float8_e4m3
    a = np.ascontiguousarray(a, dtype=np.float32)
    hi = a.astype(E4)
    lo = (a - hi.astype(np.float32)).astype(E4)
    return hi, lo


def _in_maps(x, wq, bq, wk, bk, wv, bv, wp):
    x = np.asarray(x, dtype=np.float32)
    maps = []
    xparts = []
    for b in range(B):
        xT = np.ascontiguousarray(x[b].T)                        # [D, S]
        pk = xT.reshape(KT, P, NQC, 512).transpose(2, 1, 0, 3)   # [c4,p,kt,n]
        hi, lo = _split8(pk)
        xparts.append((np.ascontiguousarray(hi), np.ascontiguousarray(lo)))
    for c in range(NCORES):
        b = c // 4
        cols = slice((c % 4) * CW, (c % 4) * CW + CW)
        m = {"xh": xparts[b][0], "xl": xparts[b][1]}
        for name, w in (("wq", wq), ("wk", wk), ("wv", wv)):
            w32 = 32.0 * np.asarray(w, np.float32)[:, cols]
            pk = w32.reshape(KT, P, CW).transpose(1, 0, 2)       # [p, kt, c]
            hi, lo = _split8(pk)
            m[name + "h"] = np.ascontiguousarray(hi)
            m[name + "l"] = np.ascontiguousarray(lo)
        wp32 = 32.0 * np.asarray(wp, np.float32)[cols, :]
        pk = wp32.reshape(HH, P, D).transpose(1, 0, 2)           # [p, hh, c]
        hi, lo = _split8(pk)
        m["wph"] = np.ascontiguousarray(hi)
        m["wpl"] = np.ascontiguousarray(lo)
        m["bq"] = np.ascontiguousarray(
            np.asarray(bq, np.float32)[cols].reshape(HH, P).T)
        m["bk"] = np.ascontiguousarray(
            np.asarray(bk, np.float32)[cols].reshape(HH, P).T)
        m["bv"] = np.ascontiguousarray(
            32.0 * np.asarray(bv, np.float32)[cols])
        maps.append(m)
    return maps


def kernel(x, wq, bq, wk, bk, wv, bv, wp, bp):
    from concourse.bass_utils import run_bass_kernel_spmd

    nc = _get_nc()
    maps = _in_maps(x, wq, bq, wk, bk, wv, bv, wp)
    res = run_bass_kernel_spmd(nc, maps, core_ids=list(range(NCORES)))
    parts = [res.results[c]["out"] for c in range(NCORES)]
    bp = np.asarray(bp, dtype=np.float32)
    full = np.empty((B, S, D), dtype=np.float32)
    for b in range(B):
        acc = parts[4 * b].astype(np.float64)
        for c in range(4 * b + 1, 4 * b + 4):
            acc += parts[c]
        full[b] = (acc + bp).astype(np.float32)
    return full



# revision 42
# speedup vs baseline: 1.2902x; 1.0754x over previous
"""Multi-head causal attention (B=2, S=2048, D=2048, H=16) on 8 TRN2 NeuronCores.

Sharding (host-side): core c in 0..7 handles batch b=c//4 and heads
4*(c%4)..4*(c%4)+4 (a 512-wide column slice of wq/wk/wv, row slice of wp).
Each core computes its 4 heads' attention and a partial output projection
[S, D] in fp16; the host sums the 4 partials per batch and adds bp.

Per-core kernel (~2e-3 rel err, dominated by the fp8/bf16 quantization):
  A) QKV projections in fp8e4 DoubleRow (two 128-deep k-tiles per matmul at
     0.5 cyc/row -> 4x fp32r FLOP rate).  x and the weights arrive from the
     host split into fp8 hi+lo pairs; computing hi*hi + hi*lo + lo*hi gives
     a ~0.13% error at 0.75x the fp32r PE time.  w is host-scaled by 32 to
     clear the e4m3 subnormal floor; q/k undo it in the ACT readout, v keeps
     it (the softmax denominator reciprocal absorbs it via ones=32).
     qT/kT/v are written bf16 straight into resident SBUF - no DRAM scratch,
     so phase B has no input DMA at all.
  B) Per head, per 512-wide q chunk j: scoresT = K_tile @ Q_chunk in
     [keys, q] layout (causal: only key tiles <= diagonal; diagonal
     sub-tiles narrowed to their live strip).  exp via ACT into bf16 pt
     tiles; Pool memsets the dead strip and affine_select zeros the
     sub-diagonal corner, keeping the PE free of mask matmuls.  ctxT[hd, q]
     accumulates in PSUM over key tiles; denominators: bf16 DVE pairwise
     pre-sums (2x packed mode) contracted by ones(=32)-matmuls, reciprocal,
     Pool partition_broadcast, one DVE multiply.  ctx is emitted as an
     fp8e4 hi/lo pair (Pool convert + DVE subtract) for phase C.
  C) Output projection in fp8 DoubleRow over head pairs (3 hi/lo terms);
     ACT/DVE undo the 32x wp scale on psum readout and store fp16.
"""
import sys
if "/opt/trn_rl_repo" not in sys.path:
    sys.path.insert(0, "/opt/trn_rl_repo")

import numpy as np

B, S, D = 2, 2048, 2048
H, HD = 16, 128
NCORES = 8
HH = 4            # heads per core
CW = HH * HD      # 512 column slice per core
P = 128
KT = D // P       # 16 contraction tiles
NQC = 4           # q chunks of 512
NKT = S // P      # 16 key tiles
SCALE = 1.0 / float(np.sqrt(HD))

_cache = {}


def _build():
    import concourse.bass as bass
    import concourse.tile as tile
    from concourse import bacc, mybir

    F32 = mybir.dt.float32
    F16 = mybir.dt.float16
    BF16 = mybir.dt.bfloat16
    F32R = mybir.dt.float32r
    AF = mybir.ActivationFunctionType
    ALU = mybir.AluOpType

    FP8 = mybir.dt.float8e4
    DR = mybir.MatmulPerfMode.DoubleRow

    nc = bacc.Bacc("TRN2", target_bir_lowering=False, debug=False, num_devices=NCORES)

    # x and the projection weights arrive host-split into fp8e4 hi+lo pairs
    # (w pre-scaled by 32 so its mass clears the e4m3 subnormal floor).
    # Layouts are PE-ready: x [chunk][p][kt][512 tokens], w [p][kt][out-col].
    xh = nc.dram_tensor("xh", [NQC, P, KT, 512], FP8, kind="ExternalInput")
    xl = nc.dram_tensor("xl", [NQC, P, KT, 512], FP8, kind="ExternalInput")
    w8 = {}
    for wn in ("wq", "wk", "wv"):
        for part in ("h", "l"):
            w8[wn + part] = nc.dram_tensor(
                wn + part, [P, KT, CW], FP8, kind="ExternalInput")
    wph = nc.dram_tensor("wph", [P, HH, D], FP8, kind="ExternalInput")
    wpl = nc.dram_tensor("wpl", [P, HH, D], FP8, kind="ExternalInput")
    # bq/bk arrive host-pre-arranged as [p, h] so the load is contiguous
    bq = nc.dram_tensor("bq", [P, HH], F32, kind="ExternalInput")
    bk = nc.dram_tensor("bk", [P, HH], F32, kind="ExternalInput")
    bv = nc.dram_tensor("bv", [CW], F32, kind="ExternalInput")   # x32 on host
    out = nc.dram_tensor("out", [S, D], F16, kind="ExternalOutput")

    with tile.TileContext(nc) as tc:
        with tc.tile_pool(name="consts", bufs=1) as consts:
            # per-head per-partition biases for qT/kT layout: [p, h]
            bq_sb = consts.tile([P, HH], F32)
            bk_sb = consts.tile([P, HH], F32)
            # denominator contraction vector: value 32 so the reciprocal
            # also undoes the 32x scale carried by v (wv was host-scaled)
            ones_f32 = consts.tile([P, 1], F32)
            nc.vector.memset(ones_f32, 32.0)
            ones_col = consts.tile([P, 1], BF16)
            nc.vector.tensor_copy(ones_col, ones_f32)
            ident_r = consts.tile([P, P], F32R)
            nc.vector.memset(ident_r.bitcast(F32), 0.0)
            nc.gpsimd.affine_select(
                out=ident_r, in_=ident_r,
                compare_op=ALU.not_equal, fill=1.0,
                base=0, channel_multiplier=1, pattern=[[-1, P]],
            )

            # qT/kT/v for all heads stay resident in SBUF in bf16: phase B
            # runs entirely out of SBUF (no DRAM round-trips, no seam DMA)
            kT_all = consts.tile([P, HH, S], BF16)
            qT_all = consts.tile([P, HH, S], BF16)
            v_all = consts.tile([P, NKT, CW], BF16)

            # ---------------- Phase A: QKV projections ----------------
            # fp8 DoubleRow: each matmul contracts TWO 128-deep k-tiles at
            # 0.5 cyc/row -> 4x fp32r throughput; the hi/lo error split costs
            # 3 terms (hi*hi + hi*lo + lo*hi) for a net 0.75x PE time
            with tc.tile_pool(name="xt_pool", bufs=6) as xt_pool, \
                 tc.tile_pool(name="w_pool", bufs=6) as w_pool, \
                 tc.tile_pool(name="aconsts", bufs=1) as aconsts, \
                 tc.tile_pool(name="psA", bufs=8, space="PSUM") as psA:

                HQ = [nc.sync, nc.scalar]
                bv_sb = aconsts.tile([P, CW], F32)
                # warm-up operand (phase-A scope only)
                warm_r = aconsts.tile([P, 256], F32R)
                nc.vector.memset(warm_r.bitcast(F32), 0.0)

                # weights on SWDGE in kt-halves (region DMAs into one tile
                # per tensor) so the first matmuls start before full arrival;
                # xt chunks stream on the two HWDGE queues in kt-quarters
                w_ts = {}
                for wname in ("wqh", "wql", "wkh", "wkl"):
                    t = w_pool.tile([P, KT, CW], FP8, tag="w", name=wname)
                    nc.gpsimd.dma_start(t[:, 0:8, :], w8[wname][:, 0:8, :])
                    nc.gpsimd.dma_start(t[:, 8:16, :], w8[wname][:, 8:16, :])
                    w_ts[wname] = t
                    if wname == "wqh":
                        nc.gpsimd.dma_start(bq_sb, bq[:])
                        nc.gpsimd.dma_start(bk_sb, bk[:])
                xt_t = {}

                def load_xt_chunk(c4):
                    for part, src, q in (("h", xh, nc.sync),
                                         ("l", xl, nc.scalar)):
                        t = xt_pool.tile([P, KT, 512], FP8, tag="xt",
                                         name=f"xt{part}_{c4}")
                        for g in range(4):
                            q.dma_start(t[:, 4 * g:4 * g + 4, :],
                                        src[c4][:, 4 * g:4 * g + 4, :])
                        xt_t[(part, c4)] = t

                load_xt_chunk(0)
                load_xt_chunk(1)
                # wv + bv on the scalar HWDGE queue: lands after the first two
                # xt chunks, before the first v sub-pass needs it
                for wname in ("wvh", "wvl"):
                    t = w_pool.tile([P, KT, CW], FP8, tag="w", name=wname)
                    nc.scalar.dma_start(t[:, 0:8, :], w8[wname][:, 0:8, :])
                    nc.scalar.dma_start(t[:, 8:16, :], w8[wname][:, 8:16, :])
                    w_ts[wname] = t
                nc.scalar.dma_start(
                    bv_sb, bass.AP(tensor=bv, offset=0, ap=[[0, P], [1, CW]])
                )
                load_xt_chunk(2)
                load_xt_chunk(3)

                # PE warm-up: spin matmuls on the const tiles while the
                # first xt/wq DMAs are in flight -- keeps the HAM clock-gate
                # warm so the first real matmuls run at full rate
                ps_warm = psA.tile([P, 256], F32, tag="psA", name="ps_warm")
                for wi in range(14):
                    nc.tensor.matmul(ps_warm, ident_r, warm_r,
                                     start=True, stop=True)

                def dr_terms(wname, c4):
                    # (lhs_tile, rhs_tile) per compensation term
                    return ((w_ts[wname + "h"], xt_t[("h", c4)]),
                            (w_ts[wname + "h"], xt_t[("l", c4)]),
                            (w_ts[wname + "l"], xt_t[("h", c4)]))

                def a_qk(wname, c4):
                    bias_sb = bq_sb if wname == "wq" else bk_sb
                    dst = qT_all if wname == "wq" else kT_all
                    pss = [psA.tile([P, 512], F32, tag="psA",
                                    name=f"psA{c4}_{h}") for h in range(HH)]
                    terms = dr_terms(wname, c4)
                    # n0-outer so each tile has one pending psum group at
                    # a time; heads stay interleaved so matmuls chase the
                    # DMA arrival order tile-by-tile
                    for n0 in (0, 256):
                        for ti, (wt, xt8) in enumerate(terms):
                            for t in range(KT // 2):
                                for h in range(HH):
                                    nc.tensor.matmul(
                                        pss[h][:, n0:n0 + 256],
                                        wt[:, 2 * t:2 * t + 2,
                                           h * HD:(h + 1) * HD],
                                        xt8[:, 2 * t:2 * t + 2,
                                            n0:n0 + 256],
                                        start=(ti == 0 and t == 0),
                                        stop=(ti == 2 and t == KT // 2 - 1),
                                        perf_mode=DR,
                                    )
                    for h in range(HH):
                        # bias-add + 1/32 w-scale undo, straight into the
                        # resident bf16 tensor
                        nc.scalar.activation(
                            dst[:, h, c4 * 512:(c4 + 1) * 512],
                            pss[h], AF.Identity,
                            bias=bias_sb[:, h:h + 1], scale=1.0 / 32.0,
                        )

                def a_v(c4):
                    terms = dr_terms("wv", c4)
                    psvs = [psA.tile([P, 512], F32, tag="psA",
                                     name=f"psV{4 * c4 + s}")
                            for s in range(4)]
                    for n0 in (0, 256):
                        for ti, (wt, xt8) in enumerate(terms):
                            for t in range(KT // 2):
                                for s in range(4):
                                    nc.tensor.matmul(
                                        psvs[s][:, n0:n0 + 256],
                                        xt8[:, 2 * t:2 * t + 2,
                                            s * P:(s + 1) * P],
                                        wt[:, 2 * t:2 * t + 2, n0:n0 + 256],
                                        start=(ti == 0 and t == 0),
                                        stop=(ti == 2 and t == KT // 2 - 1),
                                        perf_mode=DR,
                                    )
                    for s in range(4):
                        st16 = 4 * c4 + s
                        # v stays 32x-scaled (bv host-scaled to match); the
                        # denominator reciprocal undoes it via ones=32
                        nc.vector.tensor_tensor(
                            v_all[:, st16, :], psvs[s], bv_sb, ALU.add)

                # chunk-group order: xt chunk c4 dies after a_v(c4), so only
                # two chunks of xt are ever resident
                a_qk("wq", 0)
                a_qk("wq", 1)
                a_qk("wk", 0)
                a_v(0)
                a_qk("wq", 2)
                a_qk("wk", 1)
                a_v(1)
                a_qk("wq", 3)
                a_qk("wk", 2)
                a_v(2)
                a_qk("wk", 3)
                a_v(3)

            # ---------------- Phases B+C shared tiles ----------------
            with tc.tile_pool(name="bc_pool", bufs=1) as bc_pool, \
                 tc.tile_pool(name="wp_pool", bufs=2) as wp_pool:
                # ctx kept as an fp8e4 hi/lo pair (natural scale) for
                # phase C's DoubleRow matmuls; the f32 value only exists in a
                # small per-chunk scratch
                cxh = bc_pool.tile([P, HH, S], FP8)
                cxl = bc_pool.tile([P, HH, S], FP8)
                # wp prefetch: nothing else loads during phase B, so the
                # transfers are free
                wp_t = []
                for i, wsrc in enumerate((wph, wpl)):
                    t = wp_pool.tile([P, HH, D], FP8, tag="wp",
                                     name=f"wp8_{i}")
                    HQ[i % 2].dma_start(t, wsrc[:])
                    wp_t.append(t)

                # ---------------- Phase B: attention ----------------
                with tc.tile_pool(name="pT_pool", bufs=26) as pT_pool, \
                     tc.tile_pool(name="accB", bufs=2) as accB, \
                     tc.tile_pool(name="stB", bufs=2) as stB, \
                     tc.tile_pool(name="psS2", bufs=2, space="PSUM") as psS2, \
                     tc.tile_pool(name="psS", bufs=2, space="PSUM") as psS, \
                     tc.tile_pool(name="psCtx", bufs=1, space="PSUM") as psCtx, \
                     tc.tile_pool(name="psT", bufs=1, space="PSUM") as psT, \
                     tc.tile_pool(name="ppool", bufs=4) as ppool, \
                     tc.tile_pool(name="cpool", bufs=2) as cpool, \
                     nc.allow_low_precision(
                         reason="bf16/fp8 attention intermediates are "
                                "within the 2e-2 harness tolerance"):

                    # matmul/exp strip starts per diagonal sub-tile (bf16
                    # matmuls run 1 cyc/row at any width)
                    C0_MM = {0: 0, 1: P, 2: 2 * P, 3: 3 * P}

                    def b_scores(h, j):
                        # scoresT blocks + exp for q chunk j.  Off-diagonal
                        # key tiles go two-at-a-time into a 2-bank psum so a
                        # single ACT exp covers 1024 cols (ACT is the phase-B
                        # pacer; its per-instr access latency is ~185ns).
                        # Diagonal blocks: Pool memsets the dead strip and
                        # affine_select zeros the sub-diagonal corner post-exp
                        nkt = 4 * j + 4
                        qs = qT_all[:, h, j * 512:(j + 1) * 512]
                        pt_t = []
                        for pidx in range(2 * j):
                            ps2 = psS2.tile([P, 1024], F32, tag="ps2")
                            ptp = pT_pool.tile([P, 1024], BF16, tag="pt",
                                               name=f"ptp{h}_{j}_{pidx}")
                            for half in (0, 1):
                                i = 2 * pidx + half
                                nc.tensor.matmul(
                                    ps2[:, 512 * half:512 * half + 512],
                                    kT_all[:, h, i * P:(i + 1) * P], qs,
                                    start=True, stop=True,
                                )
                            nc.scalar.activation(ptp, ps2, AF.Exp,
                                                 scale=SCALE)
                            pt_t.append(ptp[:, 0:512])
                            pt_t.append(ptp[:, 512:1024])
                        for m in range(4):
                            i = 4 * j + m
                            ps_s = psS.tile([P, 512], F32, tag="ps_s")
                            c0 = C0_MM[m]
                            nc.tensor.matmul(
                                ps_s[:, c0:],
                                kT_all[:, h, i * P:(i + 1) * P], qs[:, c0:],
                                start=True, stop=True,
                            )
                            pt = pT_pool.tile([P, 512], BF16, tag="ptd",
                                              name=f"pt{h}_{j}_{i}")
                            if m > 0:
                                nc.gpsimd.memset(pt[:, :P * m], 0.0)
                                nc.scalar.activation(
                                    pt[:, P * m:], ps_s[:, P * m:],
                                    AF.Exp, scale=SCALE)
                            else:
                                nc.scalar.activation(pt, ps_s, AF.Exp,
                                                     scale=SCALE)
                            # zero the strictly-sub-diagonal corner
                            # (query col < key row)
                            nc.gpsimd.affine_select(
                                out=pt[:, P * m:P * (m + 1)],
                                in_=pt[:, P * m:P * (m + 1)],
                                compare_op=ALU.is_ge, fill=0.0,
                                base=0, channel_multiplier=-1,
                                pattern=[[1, P]],
                            )
                            pt_t.append(pt)
                        return pt_t

                    def b_tail(h, j, pt_t):
                        # ctxT and denominator PSUM accumulations, then
                        # normalize into the fp8 hi/lo ctx pair
                        nkt = 4 * j + 4
                        v_t = [v_all[:, i, h * HD:(h + 1) * HD]
                               for i in range(nkt)]
                        ps_c = psCtx.tile([P, 512], F32, tag="ps_c")
                        if j == 0:
                            # order m0, m1, m2, m3(full): m0 starts the full
                            # region, m3 full-width carries the stop (its
                            # masked cols read zeros from pt)
                            order = [(0, 0, True, False), (1, P, False, False),
                                     (2, 2 * P, False, False),
                                     (3, 0, False, True)]
                            for m, c0, st, sp in order:
                                nc.tensor.matmul(
                                    ps_c[:, c0:], v_t[m], pt_t[m][:, c0:],
                                    start=st, stop=sp,
                                )
                        else:
                            # off-diagonals first (i=0 starts full region),
                            # then m1/m2/m3 narrowed, m0 last carries stop
                            for i in range(4 * j):
                                nc.tensor.matmul(
                                    ps_c, v_t[i], pt_t[i],
                                    start=(i == 0), stop=False,
                                )
                            for m in (1, 2, 3):
                                c0 = C0_MM[m]
                                nc.tensor.matmul(
                                    ps_c[:, c0:], v_t[4 * j + m],
                                    pt_t[4 * j + m][:, c0:],
                                    start=False, stop=False,
                                )
                            nc.tensor.matmul(
                                ps_c, v_t[4 * j], pt_t[4 * j],
                                start=False, stop=True,
                            )
                        # denominator: bf16 DVE pairwise pre-sums (2x packed
                        # mode), ones(=32)-matmuls on PE contract the stream
                        npair = nkt // 2
                        ps_d = psT.tile([1, 512], F32, tag="ps_db", name="ps_d")
                        for i in range(npair):
                            pp = ppool.tile([P, 512], BF16, tag="ppair",
                                            name=f"pp{h}_{j}_{i}")
                            nc.vector.tensor_tensor(
                                pp, pt_t[2 * i], pt_t[2 * i + 1], ALU.add)
                            nc.tensor.matmul(
                                ps_d, ones_col, pp,
                                start=(i == 0), stop=(i == npair - 1),
                            )
                        rden = accB.tile([1, 512], F32, tag="rden")
                        nc.vector.reciprocal(rden, ps_d)
                        rdenb = stB.tile([P, 512], F32, tag="rdenb")
                        nc.gpsimd.partition_broadcast(rdenb, rden)
                        jsl = slice(j * 512, (j + 1) * 512)
                        ct = cpool.tile([P, 512], F32, tag="ct",
                                        name=f"ct{h}_{j}")
                        nc.vector.tensor_tensor(ct, ps_c, rdenb, ALU.mult)
                        nc.gpsimd.tensor_copy(cxh[:, h, jsl], ct)
                        nc.vector.tensor_tensor(
                            cxl[:, h, jsl], ct, cxh[:, h, jsl], ALU.subtract)

                    # tails lag scores by two chunks: the ACT exp stream of
                    # chunk j must finish before tail(j)'s last ctx matmul,
                    # so give PE two chunks of score work to chew in between
                    from collections import deque
                    pend = deque()
                    for h in range(HH):
                        for j in range(NQC):
                            pt_t = b_scores(h, j)
                            pend.append((h, j, pt_t))
                            # at a head boundary the j=3 tail needs 16 ACT
                            # exps; delay it one extra score block so the PE
                            # has work while ACT drains
                            if j == NQC - 1 and h < HH - 1:
                                continue
                            if len(pend) > 1:
                                b_tail(*pend.popleft())
                    while pend:
                        b_tail(*pend.popleft())

                # ---------------- Phase C: output projection ----------------
                # fp8 DoubleRow over hh-pairs, 3 hi/lo terms; ACT/DVE undo the
                # 32x wp scale on psum readout and emit fp16 (the host sums
                # partials in float64 anyway)
                with tc.tile_pool(name="outC", bufs=8) as outC, \
                     tc.tile_pool(name="psC", bufs=8, space="PSUM") as psC:
                    cterms = ((cxh, wp_t[0]), (cxh, wp_t[1]), (cxl, wp_t[0]))
                    for t16 in range(NKT):
                        for c4 in range(NQC):
                            ps_o = psC.tile([P, 512], F32, tag="psC",
                                            name=f"psC{t16}_{c4}")
                            for n0 in (0, 256):
                                for ti, (cx, wpt) in enumerate(cterms):
                                    for g in range(HH // 2):
                                        nc.tensor.matmul(
                                            ps_o[:, n0:n0 + 256],
                                            cx[:, 2 * g:2 * g + 2,
                                               t16 * P:(t16 + 1) * P],
                                            wpt[:, 2 * g:2 * g + 2,
                                                c4 * 512 + n0:
                                                c4 * 512 + n0 + 256],
                                            start=(ti == 0 and g == 0),
                                            stop=(ti == 2 and
                                                  g == HH // 2 - 1),
                                            perf_mode=DR,
                                        )
                            o_st = outC.tile([P, 512], F16, tag="out",
                                             name=f"out{t16}_{c4}")
                            # GPSIMD cannot read PSUM on hardware: the
                            # scale-and-convert copies alternate ACT/DVE only
                            if (t16 + c4) % 2 == 0:
                                nc.scalar.activation(o_st, ps_o, AF.Identity,
                                                     scale=1.0 / 32.0)
                            else:
                                nc.vector.tensor_scalar(
                                    o_st, ps_o, 1.0 / 32.0, None, ALU.mult)
                            [nc.sync, nc.scalar][(t16 + c4) % 2].dma_start(
                                out[t16 * P:(t16 + 1) * P,
                                    c4 * 512:(c4 + 1) * 512], o_st)

    nc.compile()
    return nc


def _get_nc():
    if "nc" not in _cache:
        _cache["nc"] = _build()
    return _cache["nc"]


def _split8(a):
    """fp8e4 hi/lo error split: a ~= hi + lo with ~0.13% residual."""
    import ml_dtypes
    E4 = ml_dtypes.float8_e4m3
    a = np.ascontiguousarray(a, dtype=np.float32)
    hi = a.astype(E4)
    lo = (a - hi.astype(np.float32)).astype(E4)
    return hi, lo


def _in_maps(x, wq, bq, wk, bk, wv, bv, wp):
    x = np.asarray(x, dtype=np.float32)
    maps = []
    xparts = []
    for b in range(B):
        xT = np.ascontiguousarray(x[b].T)                        # [D, S]
        pk = xT.reshape(KT, P, NQC, 512).transpose(2, 1, 0, 3)   # [c4,p,kt,n]
        hi, lo = _split8(pk)
        xparts.append((np.ascontiguousarray(hi), np.ascontiguousarray(lo)))
    for c in range(NCORES):
        b = c // 4
        cols = slice((c % 4) * CW, (c % 4) * CW + CW)
        m = {"xh": xparts[b][0], "xl": xparts[b][1]}
        for name, w in (("wq", wq), ("wk", wk), ("wv", wv)):
            w32 = 32.0 * np.asarray(w, np.float32)[:, cols]
            pk = w32.reshape(KT, P, CW).transpose(1, 0, 2)       # [p, kt, c]
            hi, lo = _split8(pk)
            m[name + "h"] = np.ascontiguousarray(hi)
            m[name + "l"] = np.ascontiguousarray(lo)
        wp32 = 32.0 * np.asarray(wp, np.float32)[cols, :]
        pk = wp32.reshape(HH, P, D).transpose(1, 0, 2)           # [p, hh, c]
        hi, lo = _split8(pk)
        m["wph"] = np.ascontiguousarray(hi)
        m["wpl"] = np.ascontiguousarray(lo)
        m["bq"] = np.ascontiguousarray(
            np.asarray(bq, np.float32)[cols].reshape(HH, P).T)
        m["bk"] = np.ascontiguousarray(
            np.asarray(bk, np.float32)[cols].reshape(HH, P).T)
        m["bv"] = np.ascontiguousarray(
            32.0 * np.asarray(bv, np.float32)[cols])
        maps.append(m)
    return maps


def kernel(x, wq, bq, wk, bk, wv, bv, wp, bp):
    from concourse.bass_utils import run_bass_kernel_spmd

    nc = _get_nc()
    maps = _in_maps(x, wq, bq, wk, bk, wv, bv, wp)
    res = run_bass_kernel_spmd(nc, maps, core_ids=list(range(NCORES)))
    parts = [res.results[c]["out"] for c in range(NCORES)]
    bp = np.asarray(bp, dtype=np.float32)
    full = np.empty((B, S, D), dtype=np.float32)
    for b in range(B):
        acc = parts[4 * b].astype(np.float64)
        for c in range(4 * b + 1, 4 * b + 4):
            acc += parts[c].astype(np.float64)
        full[b] = (acc + bp).astype(np.float32)
    return full


# revision 51
# speedup vs baseline: 1.2908x; 1.0004x over previous
"""Multi-head causal attention (B=2, S=2048, D=2048, H=16) on 8 TRN2 NeuronCores.

Sharding (host-side): core c in 0..7 handles batch b=c//4 and heads
4*(c%4)..4*(c%4)+4 (a 512-wide column slice of wq/wk/wv, row slice of wp).
Each core computes its 4 heads' attention and a partial output projection
[S, D] in fp16; the host sums the 4 partials per batch and adds bp.

Per-core kernel (~2e-3 rel err, dominated by the fp8/bf16 quantization):
  A) QKV projections in fp8e4 DoubleRow (two 128-deep k-tiles per matmul at
     0.5 cyc/row -> 4x fp32r FLOP rate).  x and the weights arrive from the
     host split into fp8 hi+lo pairs; computing hi*hi + hi*lo + lo*hi gives
     a ~0.13% error at 0.75x the fp32r PE time.  w is host-scaled by 32 to
     clear the e4m3 subnormal floor; q/k undo it in the ACT readout, v keeps
     it (the softmax denominator reciprocal absorbs it via ones=32).
     qT/kT/v are written bf16 straight into resident SBUF - no DRAM scratch,
     so phase B has no input DMA at all.
  B) Per head, per 512-wide q chunk j: scoresT = K_tile @ Q_chunk in
     [keys, q] layout (causal: only key tiles <= diagonal; diagonal
     sub-tiles narrowed to their live strip).  Off-diagonal key tiles go
     two-per-2-bank-psum so one ACT exp covers 1024 cols (ACT paces phase
     B; its ~185ns/instr access latency is the binding cost).  exp into
     bf16 pt tiles; Pool memsets the dead strip and affine_select zeros the
     sub-diagonal corner, keeping the PE free of mask matmuls.  ctxT[hd, q]
     accumulates in PSUM over key tiles; denominators: bf16 DVE pairwise
     pre-sums (2x packed mode) contracted by ones(=32)-matmuls, reciprocal,
     Pool partition_broadcast, one DVE multiply.  ctx is emitted as an
     fp8e4 hi/lo pair (Pool convert + DVE subtract) for phase C.
  C) Output projection in fp8 DoubleRow over head pairs (3 hi/lo terms);
     ACT/DVE undo the 32x wp scale on psum readout and store fp16.
"""
import sys
if "/opt/trn_rl_repo" not in sys.path:
    sys.path.insert(0, "/opt/trn_rl_repo")

import numpy as np

B, S, D = 2, 2048, 2048
H, HD = 16, 128
NCORES = 8
HH = 4            # heads per core
CW = HH * HD      # 512 column slice per core
P = 128
KT = D // P       # 16 contraction tiles
NQC = 4           # q chunks of 512
NKT = S // P      # 16 key tiles
SCALE = 1.0 / float(np.sqrt(HD))

_cache = {}


def _build():
    import concourse.bass as bass
    import concourse.tile as tile
    from concourse import bacc, mybir

    F32 = mybir.dt.float32
    F16 = mybir.dt.float16
    BF16 = mybir.dt.bfloat16
    F32R = mybir.dt.float32r
    AF = mybir.ActivationFunctionType
    ALU = mybir.AluOpType

    FP8 = mybir.dt.float8e4
    DR = mybir.MatmulPerfMode.DoubleRow

    nc = bacc.Bacc("TRN2", target_bir_lowering=False, debug=False, num_devices=NCORES)

    # x and the projection weights arrive host-split into fp8e4 hi+lo pairs
    # (w pre-scaled by 32 so its mass clears the e4m3 subnormal floor).
    # Layouts are PE-ready: x [chunk][p][kt][512 tokens], w [p][kt][out-col].
    xh = nc.dram_tensor("xh", [NQC, P, KT, 512], FP8, kind="ExternalInput")
    xl = nc.dram_tensor("xl", [NQC, P, KT, 512], FP8, kind="ExternalInput")
    w8 = {}
    for wn in ("wq", "wk", "wv"):
        for part in ("h", "l"):
            w8[wn + part] = nc.dram_tensor(
                wn + part, [P, KT, CW], FP8, kind="ExternalInput")
    wph = nc.dram_tensor("wph", [P, HH, D], FP8, kind="ExternalInput")
    wpl = nc.dram_tensor("wpl", [P, HH, D], FP8, kind="ExternalInput")
    # bq/bk arrive host-pre-arranged as [p, h] so the load is contiguous
    bq = nc.dram_tensor("bq", [P, HH], F32, kind="ExternalInput")
    bk = nc.dram_tensor("bk", [P, HH], F32, kind="ExternalInput")
    bv = nc.dram_tensor("bv", [CW], F32, kind="ExternalInput")   # x32 on host
    out = nc.dram_tensor("out", [S, D], F16, kind="ExternalOutput")

    with tile.TileContext(nc) as tc:
        with tc.tile_pool(name="consts", bufs=1) as consts:
            # per-head per-partition biases for qT/kT layout: [p, h]
            bq_sb = consts.tile([P, HH], F32)
            bk_sb = consts.tile([P, HH], F32)
            # denominator contraction vector: value 32 so the reciprocal
            # also undoes the 32x scale carried by v (wv was host-scaled)
            ones_f32 = consts.tile([P, 1], F32)
            nc.vector.memset(ones_f32, 32.0)
            ones_col = consts.tile([P, 1], BF16)
            nc.vector.tensor_copy(ones_col, ones_f32)
            ident_r = consts.tile([P, P], F32R)
            nc.vector.memset(ident_r.bitcast(F32), 0.0)
            nc.gpsimd.affine_select(
                out=ident_r, in_=ident_r,
                compare_op=ALU.not_equal, fill=1.0,
                base=0, channel_multiplier=1, pattern=[[-1, P]],
            )

            # qT/kT/v for all heads stay resident in SBUF in bf16: phase B
            # runs entirely out of SBUF (no DRAM round-trips, no seam DMA)
            kT_all = consts.tile([P, HH, S], BF16)
            qT_all = consts.tile([P, HH, S], BF16)
            v_all = consts.tile([P, NKT, CW], BF16)

            # ---------------- Phase A: QKV projections ----------------
            # fp8 DoubleRow: each matmul contracts TWO 128-deep k-tiles at
            # 0.5 cyc/row -> 4x fp32r throughput; the hi/lo error split costs
            # 3 terms (hi*hi + hi*lo + lo*hi) for a net 0.75x PE time
            with tc.tile_pool(name="xt_pool", bufs=6) as xt_pool, \
                 tc.tile_pool(name="w_pool", bufs=6) as w_pool, \
                 tc.tile_pool(name="aconsts", bufs=1) as aconsts, \
                 tc.tile_pool(name="psA", bufs=8, space="PSUM") as psA:

                HQ = [nc.sync, nc.scalar]
                bv_sb = aconsts.tile([P, CW], F32)
                # warm-up operand (phase-A scope only)
                warm_r = aconsts.tile([P, 256], F32R)
                nc.vector.memset(warm_r.bitcast(F32), 0.0)

                # weights on SWDGE in kt-halves (region DMAs into one tile
                # per tensor) so the first matmuls start before full arrival;
                # xt chunks stream on the two HWDGE queues in kt-quarters
                w_ts = {}
                for wname in ("wqh", "wql", "wkh", "wkl"):
                    t = w_pool.tile([P, KT, CW], FP8, tag="w", name=wname)
                    nc.gpsimd.dma_start(t[:, 0:8, :], w8[wname][:, 0:8, :])
                    nc.gpsimd.dma_start(t[:, 8:16, :], w8[wname][:, 8:16, :])
                    w_ts[wname] = t
                    if wname == "wqh":
                        nc.gpsimd.dma_start(bq_sb, bq[:])
                        nc.gpsimd.dma_start(bk_sb, bk[:])
                xt_t = {}

                def load_xt_chunk(c4):
                    for part, src, q in (("h", xh, nc.sync),
                                         ("l", xl, nc.scalar)):
                        t = xt_pool.tile([P, KT, 512], FP8, tag="xt",
                                         name=f"xt{part}_{c4}")
                        for g in range(4):
                            q.dma_start(t[:, 4 * g:4 * g + 4, :],
                                        src[c4][:, 4 * g:4 * g + 4, :])
                        xt_t[(part, c4)] = t

                load_xt_chunk(0)
                load_xt_chunk(1)
                # wv + bv on the scalar HWDGE queue: lands after the first two
                # xt chunks, before the first v sub-pass needs it
                for wname in ("wvh", "wvl"):
                    t = w_pool.tile([P, KT, CW], FP8, tag="w", name=wname)
                    nc.scalar.dma_start(t[:, 0:8, :], w8[wname][:, 0:8, :])
                    nc.scalar.dma_start(t[:, 8:16, :], w8[wname][:, 8:16, :])
                    w_ts[wname] = t
                nc.scalar.dma_start(
                    bv_sb, bass.AP(tensor=bv, offset=0, ap=[[0, P], [1, CW]])
                )
                load_xt_chunk(2)
                load_xt_chunk(3)

                # PE warm-up: spin matmuls on the const tiles while the
                # first xt/wq DMAs are in flight -- keeps the HAM clock-gate
                # warm so the first real matmuls run at full rate
                ps_warm = psA.tile([P, 256], F32, tag="psA", name="ps_warm")
                for wi in range(14):
                    nc.tensor.matmul(ps_warm, ident_r, warm_r,
                                     start=True, stop=True)

                def dr_terms(wname, c4):
                    # (lhs_tile, rhs_tile) per compensation term
                    return ((w_ts[wname + "h"], xt_t[("h", c4)]),
                            (w_ts[wname + "h"], xt_t[("l", c4)]),
                            (w_ts[wname + "l"], xt_t[("h", c4)]))

                def a_qk(wname, c4):
                    bias_sb = bq_sb if wname == "wq" else bk_sb
                    dst = qT_all if wname == "wq" else kT_all
                    pss = [psA.tile([P, 512], F32, tag="psA",
                                    name=f"psA{c4}_{h}") for h in range(HH)]
                    terms = dr_terms(wname, c4)
                    # n0-outer so each tile has one pending psum group at
                    # a time; heads stay interleaved so matmuls chase the
                    # DMA arrival order tile-by-tile
                    for n0 in (0, 256):
                        for ti, (wt, xt8) in enumerate(terms):
                            for t in range(KT // 2):
                                for h in range(HH):
                                    nc.tensor.matmul(
                                        pss[h][:, n0:n0 + 256],
                                        wt[:, 2 * t:2 * t + 2,
                                           h * HD:(h + 1) * HD],
                                        xt8[:, 2 * t:2 * t + 2,
                                            n0:n0 + 256],
                                        start=(ti == 0 and t == 0),
                                        stop=(ti == 2 and t == KT // 2 - 1),
                                        perf_mode=DR,
                                    )
                    for h in range(HH):
                        # bias-add + 1/32 w-scale undo, straight into the
                        # resident bf16 tensor
                        nc.scalar.activation(
                            dst[:, h, c4 * 512:(c4 + 1) * 512],
                            pss[h], AF.Identity,
                            bias=bias_sb[:, h:h + 1], scale=1.0 / 32.0,
                        )

                def a_v(c4):
                    terms = dr_terms("wv", c4)
                    psvs = [psA.tile([P, 512], F32, tag="psA",
                                     name=f"psV{4 * c4 + s}")
                            for s in range(4)]
                    for n0 in (0, 256):
                        for ti, (wt, xt8) in enumerate(terms):
                            for t in range(KT // 2):
                                for s in range(4):
                                    nc.tensor.matmul(
                                        psvs[s][:, n0:n0 + 256],
                                        xt8[:, 2 * t:2 * t + 2,
                                            s * P:(s + 1) * P],
                                        wt[:, 2 * t:2 * t + 2, n0:n0 + 256],
                                        start=(ti == 0 and t == 0),
                                        stop=(ti == 2 and t == KT // 2 - 1),
                                        perf_mode=DR,
                                    )
                    for s in range(4):
                        st16 = 4 * c4 + s
                        # v stays 32x-scaled (bv host-scaled to match); the
                        # denominator reciprocal undoes it via ones=32
                        nc.vector.tensor_tensor(
                            v_all[:, st16, :], psvs[s], bv_sb, ALU.add)

                # chunk-group order: xt chunk c4 dies after a_v(c4), so only
                # two chunks of xt are ever resident
                a_qk("wq", 0)
                a_qk("wq", 1)
                a_qk("wk", 0)
                a_v(0)
                a_qk("wq", 2)
                a_qk("wk", 1)
                a_v(1)
                a_qk("wq", 3)
                a_qk("wk", 2)
                a_v(2)
                a_qk("wk", 3)
                a_v(3)

            # ---------------- Phases B+C shared tiles ----------------
            with tc.tile_pool(name="bc_pool", bufs=1) as bc_pool, \
                 tc.tile_pool(name="wp_pool", bufs=2) as wp_pool:
                # ctx kept as an fp8e4 hi/lo pair (natural scale) for
                # phase C's DoubleRow matmuls; the f32 value only exists in a
                # small per-chunk scratch
                cxh = bc_pool.tile([P, HH, S], FP8)
                cxl = bc_pool.tile([P, HH, S], FP8)
                # wp prefetch: nothing else loads during phase B, so the
                # transfers are free
                wp_t = []
                for i, wsrc in enumerate((wph, wpl)):
                    t = wp_pool.tile([P, HH, D], FP8, tag="wp",
                                     name=f"wp8_{i}")
                    HQ[i % 2].dma_start(t, wsrc[:])
                    wp_t.append(t)

                # ---------------- Phase B: attention ----------------
                with tc.tile_pool(name="pT_pool", bufs=30) as pT_pool, \
                     tc.tile_pool(name="accB", bufs=2) as accB, \
                     tc.tile_pool(name="stB", bufs=2) as stB, \
                     tc.tile_pool(name="psS2", bufs=2, space="PSUM") as psS2, \
                     tc.tile_pool(name="psS", bufs=2, space="PSUM") as psS, \
                     tc.tile_pool(name="psCtx", bufs=1, space="PSUM") as psCtx, \
                     tc.tile_pool(name="psT", bufs=1, space="PSUM") as psT, \
                     tc.tile_pool(name="ppool", bufs=4) as ppool, \
                     tc.tile_pool(name="cpool", bufs=2) as cpool, \
                     nc.allow_low_precision(
                         reason="bf16/fp8 attention intermediates are "
                                "within the 2e-2 harness tolerance"):

                    # matmul/exp strip starts per diagonal sub-tile (bf16
                    # matmuls run 1 cyc/row at any width)
                    C0_MM = {0: 0, 1: P, 2: 2 * P, 3: 3 * P}

                    def b_scores(h, j):
                        # scoresT blocks + exp for q chunk j.  Off-diagonal
                        # key tiles go two-at-a-time into a 2-bank psum so a
                        # single ACT exp covers 1024 cols (ACT is the phase-B
                        # pacer; its per-instr access latency is ~185ns).
                        # Diagonal blocks: Pool memsets the dead strip and
                        # affine_select zeros the sub-diagonal corner post-exp
                        nkt = 4 * j + 4
                        qs = qT_all[:, h, j * 512:(j + 1) * 512]
                        pt_t = []
                        for pidx in range(2 * j):
                            ps2 = psS2.tile([P, 1024], F32, tag="ps2")
                            ptp = pT_pool.tile([P, 1024], BF16, tag="pt",
                                               name=f"ptp{h}_{j}_{pidx}")
                            for half in (0, 1):
                                i = 2 * pidx + half
                                nc.tensor.matmul(
                                    ps2[:, 512 * half:512 * half + 512],
                                    kT_all[:, h, i * P:(i + 1) * P], qs,
                                    start=True, stop=True,
                                )
                            nc.scalar.activation(ptp, ps2, AF.Exp,
                                                 scale=SCALE)
                            pt_t.append(ptp[:, 0:512])
                            pt_t.append(ptp[:, 512:1024])
                        for m in range(4):
                            i = 4 * j + m
                            ps_s = psS.tile([P, 512], F32, tag="ps_s")
                            c0 = C0_MM[m]
                            nc.tensor.matmul(
                                ps_s[:, c0:],
                                kT_all[:, h, i * P:(i + 1) * P], qs[:, c0:],
                                start=True, stop=True,
                            )
                            pt = pT_pool.tile([P, 512], BF16, tag="ptd",
                                              name=f"pt{h}_{j}_{i}")
                            if m > 0:
                                nc.gpsimd.memset(pt[:, :P * m], 0.0)
                                nc.scalar.activation(
                                    pt[:, P * m:], ps_s[:, P * m:],
                                    AF.Exp, scale=SCALE)
                            else:
                                nc.scalar.activation(pt, ps_s, AF.Exp,
                                                     scale=SCALE)
                            # zero the strictly-sub-diagonal corner
                            # (query col < key row)
                            nc.gpsimd.affine_select(
                                out=pt[:, P * m:P * (m + 1)],
                                in_=pt[:, P * m:P * (m + 1)],
                                compare_op=ALU.is_ge, fill=0.0,
                                base=0, channel_multiplier=-1,
                                pattern=[[1, P]],
                            )
                            pt_t.append(pt)
                        return pt_t

                    def b_tail(h, j, pt_t):
                        # ctxT and denominator PSUM accumulations, then
                        # normalize into the fp8 hi/lo ctx pair
                        nkt = 4 * j + 4
                        v_t = [v_all[:, i, h * HD:(h + 1) * HD]
                               for i in range(nkt)]
                        ps_c = psCtx.tile([P, 512], F32, tag="ps_c")
                        if j == 0:
                            # order m0, m1, m2, m3(full): m0 starts the full
                            # region, m3 full-width carries the stop (its
                            # masked cols read zeros from pt)
                            order = [(0, 0, True, False), (1, P, False, False),
                                     (2, 2 * P, False, False),
                                     (3, 0, False, True)]
                            for m, c0, st, sp in order:
                                nc.tensor.matmul(
                                    ps_c[:, c0:], v_t[m], pt_t[m][:, c0:],
                                    start=st, stop=sp,
                                )
                        else:
                            # off-diagonals first (i=0 starts full region),
                            # then m1/m2/m3 narrowed, m0 last carries stop
                            for i in range(4 * j):
                                nc.tensor.matmul(
                                    ps_c, v_t[i], pt_t[i],
                                    start=(i == 0), stop=False,
                                )
                            for m in (1, 2, 3):
                                c0 = C0_MM[m]
                                nc.tensor.matmul(
                                    ps_c[:, c0:], v_t[4 * j + m],
                                    pt_t[4 * j + m][:, c0:],
                                    start=False, stop=False,
                                )
                            nc.tensor.matmul(
                                ps_c, v_t[4 * j], pt_t[4 * j],
                                start=False, stop=True,
                            )
                        # denominator: bf16 DVE pairwise pre-sums (2x packed
                        # mode), ones(=32)-matmuls on PE contract the stream
                        npair = nkt // 2
                        ps_d = psT.tile([1, 512], F32, tag="ps_db", name="ps_d")
                        for i in range(npair):
                            pp = ppool.tile([P, 512], BF16, tag="ppair",
                                            name=f"pp{h}_{j}_{i}")
                            nc.vector.tensor_tensor(
                                pp, pt_t[2 * i], pt_t[2 * i + 1], ALU.add)
                            nc.tensor.matmul(
                                ps_d, ones_col, pp,
                                start=(i == 0), stop=(i == npair - 1),
                            )
                        rden = accB.tile([1, 512], F32, tag="rden")
                        nc.vector.reciprocal(rden, ps_d)
                        rdenb = stB.tile([P, 512], F32, tag="rdenb")
                        nc.gpsimd.partition_broadcast(rdenb, rden)
                        jsl = slice(j * 512, (j + 1) * 512)
                        ct = cpool.tile([P, 512], F32, tag="ct",
                                        name=f"ct{h}_{j}")
                        nc.vector.tensor_tensor(ct, ps_c, rdenb, ALU.mult)
                        nc.gpsimd.tensor_copy(cxh[:, h, jsl], ct)
                        nc.vector.tensor_tensor(
                            cxl[:, h, jsl], ct, cxh[:, h, jsl], ALU.subtract)

                    # tails lag scores by two chunks: the ACT exp stream of
                    # chunk j must finish before tail(j)'s last ctx matmul,
                    # so give PE two chunks of score work to chew in between
                    from collections import deque
                    pend = deque()
                    for h in range(HH):
                        for j in range(NQC):
                            pt_t = b_scores(h, j)
                            pend.append((h, j, pt_t))
                            # at a head boundary the j=3 tail needs 16 ACT
                            # exps; delay it one extra score block so the PE
                            # has work while ACT drains
                            if j == NQC - 1 and h < HH - 1:
                                continue
                            if len(pend) > 1:
                                b_tail(*pend.popleft())
                    while pend:
                        b_tail(*pend.popleft())

                # ---------------- Phase C: output projection ----------------
                # fp8 DoubleRow over hh-pairs, 3 hi/lo terms; ACT/DVE undo the
                # 32x wp scale on psum readout and emit fp16 (the host sums
                # partials in float64 anyway)
                with tc.tile_pool(name="outC", bufs=8) as outC, \
                     tc.tile_pool(name="psC", bufs=8, space="PSUM") as psC:
                    cterms = ((cxh, wp_t[0]), (cxh, wp_t[1]), (cxl, wp_t[0]))
                    for t16 in range(NKT):
                        for c4 in range(NQC):
                            ps_o = psC.tile([P, 512], F32, tag="psC",
                                            name=f"psC{t16}_{c4}")
                            for n0 in (0, 256):
                                for ti, (cx, wpt) in enumerate(cterms):
                                    for g in range(HH // 2):
                                        nc.tensor.matmul(
                                            ps_o[:, n0:n0 + 256],
                                            cx[:, 2 * g:2 * g + 2,
                                               t16 * P:(t16 + 1) * P],
                                            wpt[:, 2 * g:2 * g + 2,
                                                c4 * 512 + n0:
                                                c4 * 512 + n0 + 256],
                                            start=(ti == 0 and g == 0),
                                            stop=(ti == 2 and
                                                  g == HH // 2 - 1),
                                            perf_mode=DR,
                                        )
                            o_st = outC.tile([P, 512], F16, tag="out",
                                             name=f"out{t16}_{c4}")
                            # GPSIMD cannot read PSUM on hardware: the
                            # scale-and-convert copies alternate ACT/DVE only
                            if (t16 + c4) % 2 == 0:
                                nc.scalar.activation(o_st, ps_o, AF.Identity,
                                                     scale=1.0 / 32.0)
                            else:
                                nc.vector.tensor_scalar(
                                    o_st, ps_o, 1.0 / 32.0, None, ALU.mult)
                            [nc.sync, nc.scalar, nc.gpsimd][
                                (t16 + c4) % 3].dma_start(
                                out[t16 * P:(t16 + 1) * P,
                                    c4 * 512:(c4 + 1) * 512], o_st)

    nc.compile()
    return nc


def _get_nc():
    if "nc" not in _cache:
        _cache["nc"] = _build()
    return _cache["nc"]


def _split8(a):
    """fp8e4 hi/lo error split: a ~= hi + lo with ~0.13% residual."""
    import ml_dtypes
    E4 = ml_dtypes.float8_e4m3
    a = np.ascontiguousarray(a, dtype=np.float32)
    hi = a.astype(E4)
    lo = (a - hi.astype(np.float32)).astype(E4)
    return hi, lo


def _in_maps(x, wq, bq, wk, bk, wv, bv, wp):
    x = np.asarray(x, dtype=np.float32)
    maps = []
    xparts = []
    for b in range(B):
        xT = np.ascontiguousarray(x[b].T)                        # [D, S]
        pk = xT.reshape(KT, P, NQC, 512).transpose(2, 1, 0, 3)   # [c4,p,kt,n]
        hi, lo = _split8(pk)
        xparts.append((np.ascontiguousarray(hi), np.ascontiguousarray(lo)))
    for c in range(NCORES):
        b = c // 4
        cols = slice((c % 4) * CW, (c % 4) * CW + CW)
        m = {"xh": xparts[b][0], "xl": xparts[b][1]}
        for name, w in (("wq", wq), ("wk", wk), ("wv", wv)):
            w32 = 32.0 * np.asarray(w, np.float32)[:, cols]
            pk = w32.reshape(KT, P, CW).transpose(1, 0, 2)       # [p, kt, c]
            hi, lo = _split8(pk)
            m[name + "h"] = np.ascontiguousarray(hi)
            m[name + "l"] = np.ascontiguousarray(lo)
        wp32 = 32.0 * np.asarray(wp, np.float32)[cols, :]
        pk = wp32.reshape(HH, P, D).transpose(1, 0, 2)           # [p, hh, c]
        hi, lo = _split8(pk)
        m["wph"] = np.ascontiguousarray(hi)
        m["wpl"] = np.ascontiguousarray(lo)
        m["bq"] = np.ascontiguousarray(
            np.asarray(bq, np.float32)[cols].reshape(HH, P).T)
        m["bk"] = np.ascontiguousarray(
            np.asarray(bk, np.float32)[cols].reshape(HH, P).T)
        m["bv"] = np.ascontiguousarray(
            32.0 * np.asarray(bv, np.float32)[cols])
        maps.append(m)
    return maps


def kernel(x, wq, bq, wk, bk, wv, bv, wp, bp):
    from concourse.bass_utils import run_bass_kernel_spmd

    nc = _get_nc()
    maps = _in_maps(x, wq, bq, wk, bk, wv, bv, wp)
    res = run_bass_kernel_spmd(nc, maps, core_ids=list(range(NCORES)))
    parts = [res.results[c]["out"] for c in range(NCORES)]
    bp = np.asarray(bp, dtype=np.float32)
    full = np.empty((B, S, D), dtype=np.float32)
    for b in range(B):
        acc = parts[4 * b].astype(np.float64)
        for c in range(4 * b + 1, 4 * b + 4):
            acc += parts[c].astype(np.float64)
        full[b] = (acc + bp).astype(np.float32)
    return full


# revision 71
# speedup vs baseline: 1.3436x; 1.0410x over previous
"""Multi-head causal attention (B=2, S=2048, D=2048, H=16) on 8 TRN2 NeuronCores.

Sharding (host-side): core c in 0..7 handles batch b=c//4 and heads
4*(c%4)..4*(c%4)+4 (a 512-wide column slice of wq/wk/wv, row slice of wp).
Each core computes its 4 heads' attention and a partial output projection
[S, D] in fp16; the host sums the 4 partials per batch and adds bp.

Per-core kernel (~2e-3 rel err, dominated by the fp8/bf16 quantization).
The QKV projections (PE-heavy) are interleaved with the attention
(ACT-heavy): attention runs q-chunk-major (all heads at chunk j) as soon
as chunk j's q/k/v exist, while per-head projection sub-units for chunk
j+1 fill the PE between score blocks and tails.

  A) QKV projections in fp8e4 DoubleRow (two 128-deep k-tiles per matmul at
     0.5 cyc/row -> 4x fp32r FLOP rate).  x and the weights arrive from the
     host split into fp8 hi+lo pairs; computing hi*hi + hi*lo + lo*hi gives
     a ~0.13% error at 0.75x the fp32r PE time.  w is host-scaled by 32 to
     clear the e4m3 subnormal floor; q/k undo it in the ACT readout, v keeps
     it (the softmax denominator reciprocal absorbs it via ones=32).
     qT/kT/v are written bf16 straight into resident SBUF - no DRAM scratch,
     so phase B has no input DMA at all.
  B) Per head, per 512-wide q chunk j: scoresT = K_tile @ Q_chunk in
     [keys, q] layout (causal: only key tiles <= diagonal; diagonal
     sub-tiles narrowed to their live strip).  Off-diagonal key tiles go
     two-per-2-bank-psum so one ACT exp covers 1024 cols (ACT paces phase
     B; its ~185ns/instr access latency is the binding cost).  exp into
     bf16 pt tiles; Pool memsets the dead strip and affine_select zeros the
     sub-diagonal corner, keeping the PE free of mask matmuls.  ctxT[hd, q]
     accumulates in PSUM over key tiles; denominators: bf16 DVE pairwise
     pre-sums (2x packed mode) contracted by ones(=32)-matmuls, reciprocal,
     Pool partition_broadcast, one DVE multiply.  ctx is emitted as an
     fp8e4 hi/lo pair (Pool convert + DVE subtract) for phase C.
  C) Output projection in fp8 DoubleRow over head pairs (3 hi/lo terms);
     ACT/DVE undo the 32x wp scale on psum readout and store fp16.
"""
import sys
if "/opt/trn_rl_repo" not in sys.path:
    sys.path.insert(0, "/opt/trn_rl_repo")

import numpy as np

B, S, D = 2, 2048, 2048
H, HD = 16, 128
NCORES = 8
HH = 4            # heads per core
CW = HH * HD      # 512 column slice per core
P = 128
KT = D // P       # 16 contraction tiles
NQC = 4           # q chunks of 512
NKT = S // P      # 16 key tiles
SCALE = 1.0 / float(np.sqrt(HD))

_cache = {}


def _build():
    import concourse.bass as bass
    import concourse.tile as tile
    from concourse import bacc, mybir

    F32 = mybir.dt.float32
    F16 = mybir.dt.float16
    BF16 = mybir.dt.bfloat16
    F32R = mybir.dt.float32r
    AF = mybir.ActivationFunctionType
    ALU = mybir.AluOpType

    FP8 = mybir.dt.float8e4
    DR = mybir.MatmulPerfMode.DoubleRow

    nc = bacc.Bacc("TRN2", target_bir_lowering=False, debug=False, num_devices=NCORES)

    xh = nc.dram_tensor("xh", [NQC, P, KT, 512], FP8, kind="ExternalInput")
    xl = nc.dram_tensor("xl", [NQC, P, KT, 512], FP8, kind="ExternalInput")
    w8 = {}
    for wn in ("wq", "wk", "wv"):
        for part in ("h", "l"):
            w8[wn + part] = nc.dram_tensor(
                wn + part, [P, KT, CW], FP8, kind="ExternalInput")
    wph = nc.dram_tensor("wph", [P, HH, D], FP8, kind="ExternalInput")
    wpl = nc.dram_tensor("wpl", [P, HH, D], FP8, kind="ExternalInput")
    bq = nc.dram_tensor("bq", [P, HH], F32, kind="ExternalInput")
    bk = nc.dram_tensor("bk", [P, HH], F32, kind="ExternalInput")
    bv = nc.dram_tensor("bv", [CW], F32, kind="ExternalInput")   # x32 on host
    out = nc.dram_tensor("out", [S, D], F16, kind="ExternalOutput")

    # DVE quad pre-sums per chunk: each merges two pairs before the
    # ones-matmul, trading a cheap bf16 DVE add for a 512-cycle PE matmul
    NQUAD = {0: 1, 1: 2, 2: 3, 3: 3}

    with tile.TileContext(nc) as tc:
        with tc.tile_pool(name="consts", bufs=1) as consts:
            bq_sb = consts.tile([P, HH], F32)
            bk_sb = consts.tile([P, HH], F32)
            ones_f32 = consts.tile([P, 1], F32)
            nc.vector.memset(ones_f32, 32.0)
            ones_col = consts.tile([P, 1], BF16)
            nc.vector.tensor_copy(ones_col, ones_f32)
            ident_r = consts.tile([P, P], F32R)
            nc.vector.memset(ident_r.bitcast(F32), 0.0)
            nc.gpsimd.affine_select(
                out=ident_r, in_=ident_r,
                compare_op=ALU.not_equal, fill=1.0,
                base=0, channel_multiplier=1, pattern=[[-1, P]],
            )

            kT_all = consts.tile([P, HH, S], BF16)
            qT_all = consts.tile([P, HH, S], BF16)
            v_all = consts.tile([P, NKT, CW], BF16)

            from contextlib import ExitStack
            _bs = ExitStack()
            with _bs:
                bc_pool = _bs.enter_context(tc.tile_pool(name="bc_pool", bufs=1))
                wp_pool = _bs.enter_context(tc.tile_pool(name="wp_pool", bufs=2))
                pT_pool = _bs.enter_context(tc.tile_pool(name="pT_pool", bufs=29))
                accB = _bs.enter_context(tc.tile_pool(name="accB", bufs=1))
                stB = _bs.enter_context(tc.tile_pool(name="stB", bufs=1))
                ppool = _bs.enter_context(tc.tile_pool(name="ppool", bufs=4))
                qpool = _bs.enter_context(tc.tile_pool(name="qpool", bufs=1))
                cpool = _bs.enter_context(tc.tile_pool(name="cpool", bufs=2))
                outC2 = _bs.enter_context(tc.tile_pool(name="outC2", bufs=2))
                _bs.enter_context(nc.allow_low_precision(
                    reason="bf16/fp8 attention intermediates are within "
                           "the 2e-2 harness tolerance"))
                cxh = bc_pool.tile([P, HH, S], FP8)
                cxl = bc_pool.tile([P, HH, S], FP8)
                wp_t = [wp_pool.tile([P, HH, D], FP8, tag="wp",
                                     name=f"wp8_{i}") for i in range(2)]

                with tc.tile_pool(name="psS", bufs=4, space="PSUM") as psS, \
                     tc.tile_pool(name="psCtx", bufs=1, space="PSUM") as psCtx, \
                     tc.tile_pool(name="psT", bufs=1, space="PSUM") as psT:

                    C0_MM = {0: 0, 1: P, 2: 2 * P, 3: 3 * P}

                    def b_scores(h, j):
                        nkt = 4 * j + 4
                        qs = qT_all[:, h, j * 512:(j + 1) * 512]
                        pt_t = []
                        for i in range(nkt):
                            ps_s = psS.tile([P, 512], F32, tag="ps_s")
                            m = i - 4 * j
                            c0 = C0_MM[m] if m >= 0 else 0
                            nc.tensor.matmul(
                                ps_s[:, c0:],
                                kT_all[:, h, i * P:(i + 1) * P], qs[:, c0:],
                                start=True, stop=True,
                            )
                            pt = pT_pool.tile([P, 512], BF16, tag="pt",
                                              name=f"pt{h}_{j}_{i}")
                            if m > 0:
                                nc.gpsimd.memset(pt[:, :P * m], 0.0)
                                nc.scalar.activation(
                                    pt[:, P * m:], ps_s[:, P * m:],
                                    AF.Exp, scale=SCALE)
                            else:
                                nc.scalar.activation(pt, ps_s, AF.Exp,
                                                     scale=SCALE)
                            if m >= 0:
                                nc.gpsimd.affine_select(
                                    out=pt[:, P * m:P * (m + 1)],
                                    in_=pt[:, P * m:P * (m + 1)],
                                    compare_op=ALU.is_ge, fill=0.0,
                                    base=0, channel_multiplier=-1,
                                    pattern=[[1, P]],
                                )
                            pt_t.append(pt)
                        return pt_t

                    def b_tail(h, j, pt_t):
                        nkt = 4 * j + 4
                        v_t = [v_all[:, i, h * HD:(h + 1) * HD]
                               for i in range(nkt)]
                        ps_c = psCtx.tile([P, 512], F32, tag="ps_c")
                        if j == 0:
                            order = [(0, 0, True, False), (1, P, False, False),
                                     (2, 2 * P, False, False),
                                     (3, 0, False, True)]
                            for m, c0, st, sp in order:
                                nc.tensor.matmul(
                                    ps_c[:, c0:], v_t[m], pt_t[m][:, c0:],
                                    start=st, stop=sp,
                                )
                        else:
                            for i in range(4 * j):
                                nc.tensor.matmul(
                                    ps_c, v_t[i], pt_t[i],
                                    start=(i == 0), stop=False,
                                )
                            for m in (1, 2, 3):
                                c0 = C0_MM[m]
                                nc.tensor.matmul(
                                    ps_c[:, c0:], v_t[4 * j + m],
                                    pt_t[4 * j + m][:, c0:],
                                    start=False, stop=False,
                                )
                            nc.tensor.matmul(
                                ps_c, v_t[4 * j], pt_t[4 * j],
                                start=False, stop=True,
                            )
                        # denominator: bf16 DVE pairs, optional DVE quads,
                        # ones(=32)-matmuls contract the stream into ps_d
                        npair = nkt // 2
                        nquad = NQUAD[j]
                        nmm = npair - nquad
                        ps_d = psT.tile([1, 512], F32, tag="ps_db",
                                        name="ps_d")
                        k = 0
                        prev = None
                        for i in range(npair):
                            pp = ppool.tile([P, 512], BF16, tag="ppair",
                                            name=f"pp{h}_{j}_{i}")
                            nc.vector.tensor_tensor(
                                pp, pt_t[2 * i], pt_t[2 * i + 1], ALU.add)
                            if i < 2 * nquad:
                                if i % 2 == 0:
                                    prev = pp
                                    continue
                                src = qpool.tile([P, 512], BF16, tag="quad",
                                                 name=f"qq{h}_{j}_{i}")
                                nc.vector.tensor_tensor(
                                    src, prev, pp, ALU.add)
                            else:
                                src = pp
                            nc.tensor.matmul(
                                ps_d, ones_col, src,
                                start=(k == 0), stop=(k == nmm - 1),
                            )
                            k += 1
                        rden = accB.tile([1, 512], F32, tag="rden")
                        nc.vector.reciprocal(rden, ps_d)
                        rdenb = stB.tile([P, 512], F32, tag="rdenb")
                        nc.gpsimd.partition_broadcast(rdenb, rden)
                        jsl = slice(j * 512, (j + 1) * 512)
                        ct = cpool.tile([P, 512], F32, tag="ct",
                                        name=f"ct{h}_{j}")
                        if h == HH - 1 and j == NQC - 1:
                            # final tail gates the output projection: write
                            # cxh straight from the psum multiply (DVE fp8
                            # convert-on-write) so C's hi-term matmuls start
                            # without waiting the Pool-convert chain
                            nc.vector.tensor_tensor(
                                cxh[:, h, jsl], ps_c, rdenb, ALU.mult)
                            nc.vector.tensor_tensor(ct, ps_c, rdenb,
                                                    ALU.mult)
                        else:
                            nc.vector.tensor_tensor(ct, ps_c, rdenb,
                                                    ALU.mult)
                            nc.gpsimd.tensor_copy(cxh[:, h, jsl], ct)
                        nc.vector.tensor_tensor(
                            cxl[:, h, jsl], ct, cxh[:, h, jsl], ALU.subtract)

                    # ---- projections, interleaved with the attention ----
                    with tc.tile_pool(name="xt_pool", bufs=4) as xt_pool, \
                         tc.tile_pool(name="w_pool", bufs=6) as w_pool, \
                         tc.tile_pool(name="aconsts", bufs=1) as aconsts, \
                         tc.tile_pool(name="psA", bufs=2,
                                      space="PSUM") as psA:

                        HQ = [nc.sync, nc.scalar]
                        bv_sb = aconsts.tile([P, CW], F32)
                        warm_r = aconsts.tile([P, 256], F32R)
                        nc.vector.memset(warm_r.bitcast(F32), 0.0)

                        # DMA plan matches prologue consumption order
                        # (wq -> wk -> wv): wq on SWDGE (gens start at t=0),
                        # wk on the sync HWDGE queue right behind xt0-hi
                        # (SWDGE's serial ~1us descriptor gens would land it
                        # too late), wv on scalar behind xt0-lo
                        w_ts = {}
                        for wname in ("wqh", "wql"):
                            t = w_pool.tile([P, KT, CW], FP8, tag="w",
                                            name=wname)
                            nc.gpsimd.dma_start(t[:, 0:8, :],
                                                w8[wname][:, 0:8, :])
                            nc.gpsimd.dma_start(t[:, 8:16, :],
                                                w8[wname][:, 8:16, :])
                            w_ts[wname] = t
                        nc.gpsimd.dma_start(bq_sb, bq[:])
                        nc.gpsimd.dma_start(bk_sb, bk[:])
                        xt_t = {}

                        def load_xt_chunk(c4, engs=None):
                            engs = engs or (nc.sync, nc.scalar)
                            for part, src, q in (("h", xh, engs[0]),
                                                 ("l", xl, engs[1])):
                                t = xt_pool.tile([P, KT, 512], FP8, tag="xt",
                                                 name=f"xt{part}_{c4}")
                                for g in range(4):
                                    q.dma_start(
                                        t[:, 4 * g:4 * g + 4, :],
                                        src[c4][:, 4 * g:4 * g + 4, :])
                                xt_t[(part, c4)] = t

                        load_xt_chunk(0)
                        for wname, q in (("wkh", nc.sync), ("wkl", nc.sync),
                                         ("wvh", nc.scalar),
                                         ("wvl", nc.scalar)):
                            t = w_pool.tile([P, KT, CW], FP8, tag="w",
                                            name=wname)
                            q.dma_start(t[:, 0:8, :], w8[wname][:, 0:8, :])
                            q.dma_start(t[:, 8:16, :], w8[wname][:, 8:16, :])
                            w_ts[wname] = t
                        nc.scalar.dma_start(
                            bv_sb,
                            bass.AP(tensor=bv, offset=0, ap=[[0, P], [1, CW]])
                        )
                        # later chunks ride the SWDGE queue: its serial
                        # ~1us descriptor-gens keep their transfers behind
                        # the prologue-critical wq/wk/wv on the shared DMA
                        load_xt_chunk(1, engs=(nc.gpsimd, nc.gpsimd))
                        load_xt_chunk(2, engs=(nc.gpsimd, nc.gpsimd))
                        load_xt_chunk(3, engs=(nc.gpsimd, nc.gpsimd))
                        for i, wsrc in enumerate((wph, wpl)):
                            HQ[i % 2].dma_start(wp_t[i], wsrc[:])

                        ps_warm = psA.tile([P, 256], F32, tag="psA",
                                           name="ps_warm")
                        for wi in range(14):
                            nc.tensor.matmul(ps_warm, ident_r, warm_r,
                                             start=True, stop=True)

                        def dr_terms(wname, c4):
                            return ((w_ts[wname + "h"], xt_t[("h", c4)]),
                                    (w_ts[wname + "h"], xt_t[("l", c4)]),
                                    (w_ts[wname + "l"], xt_t[("h", c4)]))

                        def aq_unit(wname, c4, h):
                            bias_sb = bq_sb if wname == "wq" else bk_sb
                            dst = qT_all if wname == "wq" else kT_all
                            ps = psA.tile([P, 512], F32, tag="psA",
                                          name=f"ps_{wname}{c4}_{h}")
                            terms = dr_terms(wname, c4)
                            for n0 in (0, 256):
                                for ti, (wt, xt8) in enumerate(terms):
                                    for t in range(KT // 2):
                                        nc.tensor.matmul(
                                            ps[:, n0:n0 + 256],
                                            wt[:, 2 * t:2 * t + 2,
                                               h * HD:(h + 1) * HD],
                                            xt8[:, 2 * t:2 * t + 2,
                                                n0:n0 + 256],
                                            start=(ti == 0 and t == 0),
                                            stop=(ti == 2 and
                                                  t == KT // 2 - 1),
                                            perf_mode=DR,
                                        )
                            nc.scalar.activation(
                                dst[:, h, c4 * 512:(c4 + 1) * 512],
                                ps, AF.Identity,
                                bias=bias_sb[:, h:h + 1], scale=1.0 / 32.0,
                            )

                        def av_unit(c4, s):
                            st16 = 4 * c4 + s
                            ps = psA.tile([P, 512], F32, tag="psA",
                                          name=f"psV{st16}")
                            terms = dr_terms("wv", c4)
                            for n0 in (0, 256):
                                for ti, (wt, xt8) in enumerate(terms):
                                    for t in range(KT // 2):
                                        nc.tensor.matmul(
                                            ps[:, n0:n0 + 256],
                                            xt8[:, 2 * t:2 * t + 2,
                                                s * P:(s + 1) * P],
                                            wt[:, 2 * t:2 * t + 2,
                                               n0:n0 + 256],
                                            start=(ti == 0 and t == 0),
                                            stop=(ti == 2 and
                                                  t == KT // 2 - 1),
                                            perf_mode=DR,
                                        )
                            nc.vector.tensor_tensor(
                                v_all[:, st16, :], ps, bv_sb, ALU.add)

                        def chunk_units(c4):
                            us = [lambda h=h: aq_unit("wq", c4, h)
                                  for h in range(HH)]
                            us += [lambda h=h: aq_unit("wk", c4, h)
                                   for h in range(HH)]
                            us += [lambda s=s: av_unit(c4, s)
                                   for s in range(4)]
                            return us

                        # prologue: chunk 0's q and k units, then its score
                        # blocks (feeds ACT), then the v units, then tails
                        # interleaved with chunk-1 units
                        units0 = chunk_units(0)
                        for u in units0[:8]:
                            u()
                        pts0 = [b_scores(h, 0) for h in range(HH)]
                        for u in units0[8:]:
                            u()
                        units = chunk_units(1)
                        for h in range(HH):
                            for u in units[3 * h:3 * h + 3]:
                                u()
                            b_tail(h, 0, pts0[h])
                        # slices 1..2: attention on chunk j with chunk j+1's
                        # projection units spread between the score blocks
                        for j in (1, 2):
                            units = chunk_units(j + 1)
                            for h in range(HH):
                                pt_t = b_scores(h, j)
                                for u in units[3 * h:3 * h + 3]:
                                    u()
                                b_tail(h, j, pt_t)
                        # head 0's last score block issues before the A pools
                        # close: ACT gets a head start on the final exp
                        # stream, which gates the output projection
                        pts30 = b_scores(0, 3)

                    # slice 3: pure attention, ACT and PE self-balance
                    b_tail(0, 3, pts30)
                    for h in range(1, HH):
                        pt_t = b_scores(h, 3)
                        b_tail(h, 3, pt_t)

                    cterms = ((cxh, wp_t[0]), (cxh, wp_t[1]), (cxl, wp_t[0]))

                    def c_tile(t16, c4, pspool, ostpool):
                        ps_o = pspool.tile([P, 512], F32, tag="psC",
                                           name=f"psC{t16}_{c4}")
                        for n0 in (0, 256):
                            for ti, (cx, wpt) in enumerate(cterms):
                                for g in range(HH // 2):
                                    nc.tensor.matmul(
                                        ps_o[:, n0:n0 + 256],
                                        cx[:, 2 * g:2 * g + 2,
                                           t16 * P:(t16 + 1) * P],
                                        wpt[:, 2 * g:2 * g + 2,
                                            c4 * 512 + n0:
                                            c4 * 512 + n0 + 256],
                                        start=(ti == 0 and g == 0),
                                        stop=(ti == 2 and g == HH // 2 - 1),
                                        perf_mode=DR,
                                    )
                        o_st = ostpool.tile([P, 512], F16, tag="out",
                                            name=f"out{t16}_{c4}")
                        if (t16 + c4) % 2 == 0:
                            nc.scalar.activation(o_st, ps_o, AF.Identity,
                                                 scale=1.0 / 32.0)
                        else:
                            nc.vector.tensor_scalar(
                                o_st, ps_o, 1.0 / 32.0, None, ALU.mult)
                        [nc.sync, nc.scalar, nc.gpsimd][
                            (t16 + c4) % 3].dma_start(
                            out[t16 * P:(t16 + 1) * P,
                                c4 * 512:(c4 + 1) * 512], o_st)

                    # first two projection tiles run on the banks the
                    # (closed) psA pool freed after slice 2 -- no wait on
                    # the B psum pools' teardown barrier
                    with tc.tile_pool(name="psC2", bufs=2,
                                      space="PSUM") as psC2:
                        for idx in range(6):
                            c_tile(idx // 4, idx % 4, psC2, outC2)

                # ---------------- output projection ----------------
                with tc.tile_pool(name="outC", bufs=8) as outC, \
                     tc.tile_pool(name="psC", bufs=6, space="PSUM") as psC:
                    for t16 in range(NKT):
                        for c4 in range(NQC):
                            if t16 * 4 + c4 < 6:
                                continue
                            c_tile(t16, c4, psC, outC)

    nc.compile()
    return nc


def _get_nc():
    if "nc" not in _cache:
        _cache["nc"] = _build()
    return _cache["nc"]


def _split8(a):
    """fp8e4 hi/lo error split: a ~= hi + lo with ~0.13% residual."""
    import ml_dtypes
    E4 = ml_dtypes.float8_e4m3
    a = np.ascontiguousarray(a, dtype=np.float32)
    hi = a.astype(E4)
    lo = (a - hi.astype(np.float32)).astype(E4)
    return hi, lo


def _in_maps(x, wq, bq, wk, bk, wv, bv, wp):
    x = np.asarray(x, dtype=np.float32)
    maps = []
    xparts = []
    for b in range(B):
        xT = np.ascontiguousarray(x[b].T)                        # [D, S]
        pk = xT.reshape(KT, P, NQC, 512).transpose(2, 1, 0, 3)   # [c4,p,kt,n]
        hi, lo = _split8(pk)
        xparts.append((np.ascontiguousarray(hi), np.ascontiguousarray(lo)))
    for c in range(NCORES):
        b = c // 4
        cols = slice((c % 4) * CW, (c % 4) * CW + CW)
        m = {"xh": xparts[b][0], "xl": xparts[b][1]}
        for name, w in (("wq", wq), ("wk", wk), ("wv", wv)):
            w32 = 32.0 * np.asarray(w, np.float32)[:, cols]
            pk = w32.reshape(KT, P, CW).transpose(1, 0, 2)       # [p, kt, c]
            hi, lo = _split8(pk)
            m[name + "h"] = np.ascontiguousarray(hi)
            m[name + "l"] = np.ascontiguousarray(lo)
        wp32 = 32.0 * np.asarray(wp, np.float32)[cols, :]
        pk = wp32.reshape(HH, P, D).transpose(1, 0, 2)           # [p, hh, c]
        hi, lo = _split8(pk)
        m["wph"] = np.ascontiguousarray(hi)
        m["wpl"] = np.ascontiguousarray(lo)
        m["bq"] = np.ascontiguousarray(
            np.asarray(bq, np.float32)[cols].reshape(HH, P).T)
        m["bk"] = np.ascontiguousarray(
            np.asarray(bk, np.float32)[cols].reshape(HH, P).T)
        m["bv"] = np.ascontiguousarray(
            32.0 * np.asarray(bv, np.float32)[cols])
        maps.append(m)
    return maps


def kernel(x, wq, bq, wk, bk, wv, bv, wp, bp):
    from concourse.bass_utils import run_bass_kernel_spmd

    nc = _get_nc()
    maps = _in_maps(x, wq, bq, wk, bk, wv, bv, wp)
    res = run_bass_kernel_spmd(nc, maps, core_ids=list(range(NCORES)))
    parts = [res.results[c]["out"] for c in range(NCORES)]
    bp = np.asarray(bp, dtype=np.float32)
    full = np.empty((B, S, D), dtype=np.float32)
    for b in range(B):
        acc = parts[4 * b].astype(np.float64)
        for c in range(4 * b + 1, 4 * b + 4):
            acc += parts[c].astype(np.float64)
        full[b] = (acc + bp).astype(np.float32)
    return full
